# revision 1
# baseline (speedup 1.0000x reference)
"""Trainium2 Bass kernel for nn_Cross_attention_dl_91061896610498.

Three dense self-attentions (no 1/sqrt(d) scaling -> logits std ~22-32,
softmax is near-one-hot, so the Q/K/score path and the stage-1 V/AV path
need fp32-grade accuracy).  Matmuls on those paths run as fp16 hi/lo
pair products (3 full-rate matmuls emulate an fp32 matmul); stage-2
V/AV runs single fp16 (its error is not amplified by a later softmax).

Sharding: 8 cores = 4 batch elements x 2 query-halves.  Each core
computes stage 1 fully for its batch element (redundant with its pair
core, avoids any collectives) and stage 2 for its query half.  The host
rolls the sequence axis per core so "my query half" is always rows
[0:1024) on device, keeping the program SPMD-identical; softmax over
keys is permutation invariant so the rolled result matches.
"""

import numpy as np

import concourse.bass as bass
import concourse.mybir as mybir
from concourse.tile import TileContext
from concourse.bass_utils import run_bass_kernel_spmd

F16 = mybir.dt.float16
F32 = mybir.dt.float32
AF = mybir.ActivationFunctionType
ALU = mybir.AluOpType
AX = mybir.AxisListType

D1, D2, B, S = 512, 1024, 4, 2048
SH = S // 2          # per-core query half
QT = 128             # query tile
NQ1 = S // QT        # stage-1 q tiles (16)
NQ2 = SH // QT       # stage-2 q tiles (8)
NC1 = D1 // 128      # 4 partition chunks of D1
NC2 = D2 // 128      # 8 partition chunks of D2
NKC = S // 128       # 16 key chunks
NSC = S // 512       # 4 moving chunks over S

_CACHED = {}


def _split16(a):
    hi = a.astype(np.float16)
    lo = (a.astype(np.float32) - hi.astype(np.float32)).astype(np.float16)
    return hi, lo


def _fix_excess_waits(nc, max_waits=1):
    """walrus in this env accepts only 1 sync-wait per instruction; move
    excess waits onto preceding same-engine NOPs."""
    ctr = 0
    for fn in nc.m.functions:
        for blk in fn.blocks:
            insts = blk.bb.instructions if hasattr(blk, "bb") else blk.instructions
            new = []
            changed = False
            for inst in insts:
                si = inst.sync_info
                waits = list(si.on_wait) if (si is not None and si.on_wait) else []
                if len(waits) > max_waits:
                    excess, keep = waits[:-max_waits], waits[-max_waits:]
                    while excess:
                        chunk, excess = excess[:max_waits], excess[max_waits:]
                        ctr += 1
                        nop = mybir.InstNoOp(name=f"I-waitfix-{ctr}", engine=inst.engine)
                        nop.sync_info = mybir.SyncInfo(on_wait=chunk, on_update=[])
                        new.append(nop)
                    inst.sync_info = mybir.SyncInfo(
                        on_wait=keep,
                        on_update=list(si.on_update) if si.on_update else [],
                    )
                    changed = True
                new.append(inst)
            if changed:
                if hasattr(blk, "bb"):
                    blk.bb.instructions = new
                else:
                    blk.instructions = new
    return ctr


def _load_pair(nc, pool, dram_hi, dram_lo, nrows, ncols, tag):
    nt = nrows // 128
    his, los = [], []
    for i in range(nt):
        th = pool.tile([128, ncols], F16, tag=f"{tag}_h{i}")
        tl = pool.tile([128, ncols], F16, tag=f"{tag}_l{i}")
        nc.sync.dma_start(out=th[:], in_=dram_hi[i * 128:(i + 1) * 128, :])
        nc.sync.dma_start(out=tl[:], in_=dram_lo[i * 128:(i + 1) * 128, :])
        his.append(th)
        los.append(tl)
    return his, los


def _pair_mms(nc, psum, lhs_pair, rhs_pair, start, stop=False):
    """Accumulate (lhs_hi+lhs_lo).T @ (rhs_hi+rhs_lo) into psum (lo*lo dropped)."""
    lh, ll = lhs_pair
    rh, rl = rhs_pair
    nc.tensor.matmul(psum, lh, rh, start=start, stop=False)
    nc.tensor.matmul(psum, lh, rl, start=False, stop=False)
    nc.tensor.matmul(psum, ll, rh, start=False, stop=stop)


def _build():
    import concourse.tile_utils as tile_utils
    tile_utils.max_sbuf_usage = 204 * 1024

    nc = bass.Bass("TRN2", target_bir_lowering=False, debug=False)

    def din(name, shape, dt=F16):
        return nc.dram_tensor(name, shape, dt, kind="ExternalInput")

    xt_hi, xt_lo = din("xt_hi", [D1, S]), din("xt_lo", [D1, S])
    yt_hi, yt_lo = din("yt_hi", [D1, S]), din("yt_lo", [D1, S])
    w1 = {t: (din(f"w1{t}_hi", [D1, D1]), din(f"w1{t}_lo", [D1, D1])) for t in "qkv"}
    w2q = (din("w2q_hi", [D2, D2]), din("w2q_lo", [D2, D2]))
    w2k = (din("w2k_hi", [D2, D2]), din("w2k_lo", [D2, D2]))
    w2v_hi = din("w2v_hi", [D2, D2])
    b1q = din("b1q", [128, NC1], F32)
    b1k = din("b1k", [128, NC1], F32)
    b2q = din("b2q", [128, NC2], F32)
    b2k = din("b2k", [128, NC2], F32)
    b1v_hi, b1v_lo = din("b1v_hi", [1, D1]), din("b1v_lo", [1, D1])
    b2v_hi, b2v_lo = din("b2v_hi", [1, D2]), din("b2v_lo", [1, D2])
    ones1 = din("ones1", [1, 128])
    wres = din("wres", [128, 2], F32)  # col0: weight2 (x1 resid), col1: weight1

    out = nc.dram_tensor("out", [SH, D2], F32, kind="ExternalOutput")

    x1t_hi = nc.dram_tensor("x1t_hi", [D1, S], F16)
    x1t_lo = nc.dram_tensor("x1t_lo", [D1, S], F16)
    y1t_hi = nc.dram_tensor("y1t_hi", [D1, S], F16)
    y1t_lo = nc.dram_tensor("y1t_lo", [D1, S], F16)
    ttd = [(x1t_hi, x1t_lo), (y1t_hi, y1t_lo)]  # tempT row-chunks: dc<4 -> x1, else y1

    with TileContext(nc) as tc:
        with tc.tile_pool(name="const", bufs=1) as cp:
            b1q_sb = cp.tile([128, NC1], F32, tag="b1q")
            b1k_sb = cp.tile([128, NC1], F32, tag="b1k")
            b2q_sb = cp.tile([128, NC2], F32, tag="b2q")
            b2k_sb = cp.tile([128, NC2], F32, tag="b2k")
            b1v_sb = (cp.tile([1, D1], F16, name="b1vh", tag="b1vh"), cp.tile([1, D1], F16, name="b1vl", tag="b1vl"))
            b2v_sb = (cp.tile([1, D2], F16, name="b2vh", tag="b2vh"), cp.tile([1, D2], F16, name="b2vl", tag="b2vl"))
            ones_sb = cp.tile([1, 128], F16, tag="ones1")
            wres_sb = cp.tile([128, 2], F32, tag="wres")
            for sb, dr in [(b1q_sb, b1q), (b1k_sb, b1k), (b2q_sb, b2q), (b2k_sb, b2k),
                           (b1v_sb[0], b1v_hi), (b1v_sb[1], b1v_lo),
                           (b2v_sb[0], b2v_hi), (b2v_sb[1], b2v_lo),
                           (ones_sb, ones1), (wres_sb, wres)]:
                nc.sync.dma_start(out=sb[:], in_=dr[:])

            # ---------------- stage 1 ----------------
            with tc.tile_pool(name="acts", bufs=1) as actp:
                xt = _load_pair(nc, actp, xt_hi, xt_lo, D1, S, "xt")
                yt = _load_pair(nc, actp, yt_hi, yt_lo, D1, S, "yt")
                w1sb = {t: _load_pair(nc, actp, w1[t][0], w1[t][1], D1, D1, f"w1{t}")
                        for t in "qkv"}
                for ti, (src, resid, wcol, o_hi, o_lo) in enumerate([
                        (xt, yt, 0, x1t_hi, x1t_lo),
                        (yt, xt, 1, y1t_hi, y1t_lo)]):
                    _stage1_attn(nc, tc, ti, src, resid, wcol, o_hi, o_lo,
                                 w1sb, b1q_sb, b1k_sb, b1v_sb, ones_sb, wres_sb)

            # ---------------- stage 2 ----------------
            _stage2(nc, tc, ttd, w2q, w2k, w2v_hi,
                    b2q_sb, b2k_sb, b2v_sb, ones_sb, out)

    _fix_excess_waits(nc)
    return nc


def _softmax_ptiles(nc, pp1, pp2, wkp, sps_h, tag, pair):
    """negmax -> exp (+row sums) -> fp16 (pair) split -> transposed halves.

    sps_h: two [128, S//2] psum tiles (score halves).  Returns
    (pth_halves, ptl_halves, recip_l): pth_halves[h] is a
    [128, NKC//2, 128] tile of transposed probabilities for key half h.
    """
    # Each key-half is softmaxed with its OWN shift m_h so its exp/split/
    # transpose/AV chain starts as soon as that half's scores land; the two
    # partial AVs are merged at evacuation with c_h = e^{m_h - m} / l.
    nm = [wkp.tile([128, 1], F32, name=f"nm{tag}{h}", tag=f"nm{tag}{h}") for h in range(2)]
    ls = [wkp.tile([128, 1], F32, name=f"ls{tag}{h}", tag=f"ls{tag}{h}") for h in range(2)]
    pth_halves, ptl_halves = [], []
    for h in range(2):
        nc.vector.reduce_max(nm[h][:], sps_h[h][:], axis=AX.X, negate=True)
        pf = pp1.tile([128, S // 2], F32, tag=f"pf{tag}")
        nc.scalar.activation(pf[:], sps_h[h][:], AF.Exp,
                             bias=nm[h][:, 0:1], accum_out=ls[h][:])
        p_hi = pp1.tile([128, S // 2], F16, tag=f"phi{tag}")
        nc.scalar.copy(p_hi[:], pf[:])
        pth = pp2.tile([128, NKC // 2, 128], F16, tag=f"pth{tag}")
        nc.sync.dma_start_transpose(pth[:], p_hi[:])
        pth_halves.append(pth)
        if pair:
            p_lo = pp1.tile([128, S // 2], F16, tag=f"plo{tag}")
            nc.vector.tensor_tensor(p_lo[:], pf[:], p_hi[:], op=ALU.subtract)
            ptl = pp2.tile([128, NKC // 2, 128], F16, tag=f"ptl{tag}")
            nc.sync.dma_start_transpose(ptl[:], p_lo[:])
            ptl_halves.append(ptl)
    negm = wkp.tile([128, 1], F32, tag=f"negm{tag}")
    nc.vector.tensor_tensor(negm[:], nm[0][:], nm[1][:], op=ALU.min)
    sh = []
    lw = [wkp.tile([128, 1], F32, name=f"lw{tag}{h}", tag=f"lw{tag}{h}") for h in range(2)]
    for h in range(2):
        d = wkp.tile([128, 1], F32, name=f"d{tag}{h}", tag=f"d{tag}{h}")
        nc.vector.tensor_tensor(d[:], negm[:], nm[h][:], op=ALU.subtract)  # m_h - m <= 0
        s = wkp.tile([128, 1], F32, name=f"sh{tag}{h}", tag=f"sh{tag}{h}")
        nc.scalar.activation(s[:], d[:], AF.Exp)
        sh.append(s)
        nc.vector.tensor_tensor(lw[h][:], ls[h][:], s[:], op=ALU.mult)
    lsum = wkp.tile([128, 1], F32, tag=f"lsum{tag}")
    nc.vector.tensor_tensor(lsum[:], lw[0][:], lw[1][:], op=ALU.add)
    rl = wkp.tile([128, 1], F32, tag=f"rl{tag}")
    nc.vector.reciprocal(rl[:], lsum[:])
    c = []
    for h in range(2):
        ch = wkp.tile([128, 1], F32, name=f"c{tag}{h}", tag=f"c{tag}{h}")
        nc.vector.tensor_tensor(ch[:], sh[h][:], rl[:], op=ALU.mult)
        c.append(ch)
    return pth_halves, ptl_halves, c


def _stage1_attn(nc, tc, ti, src, resid, wcol, o_hi, o_lo,
                 w1sb, b1q_sb, b1k_sb, b1v_sb, ones_sb, wres_sb):
    src_hi, src_lo = src
    resid_hi, resid_lo = resid
    with (tc.tile_pool(name=f"kv{ti}", bufs=1) as kvp,
          tc.tile_pool(name=f"wk{ti}", bufs=2) as wkp,
          tc.tile_pool(name=f"pa{ti}", bufs=1) as ptp1,
          tc.tile_pool(name=f"pt{ti}", bufs=2) as ptp2,
          tc.tile_pool(name=f"ps{ti}", bufs=4, space="PSUM") as pp,
          tc.tile_pool(name=f"sc{ti}", bufs=2, space="PSUM") as scp):
        # K^T pair [ec][128, S]
        kt_hi, kt_lo = [], []
        for ec in range(NC1):
            kh = kvp.tile([128, S], F16, tag=f"kth{ec}")
            kl = kvp.tile([128, S], F16, tag=f"ktl{ec}")
            for sc in range(NSC):
                ssl = slice(sc * 512, (sc + 1) * 512)
                ps = pp.tile([128, 512], F32, tag="ps")
                for dc in range(NC1):
                    _pair_mms(nc, ps[:],
                              (w1sb["k"][0][dc][:, ec * 128:(ec + 1) * 128],
                               w1sb["k"][1][dc][:, ec * 128:(ec + 1) * 128]),
                              (src_hi[dc][:, ssl], src_lo[dc][:, ssl]),
                              start=(dc == 0))
                kf = wkp.tile([128, 512], F32, tag="kevac")
                nc.vector.tensor_scalar(kf[:], ps[:], b1k_sb[:, ec:ec + 1], None, op0=ALU.add)
                nc.vector.tensor_copy(kh[:, ssl], kf[:])
                nc.vector.tensor_tensor(kl[:, ssl], kf[:], kh[:, ssl], op=ALU.subtract)
            kt_hi.append(kh)
            kt_lo.append(kl)

        # V pair [kc][128, D1] natural layout; bias via rank-1 ones x b1v
        v_hi, v_lo = [], []
        for kc in range(NKC):
            vh = kvp.tile([128, D1], F16, tag=f"vh{kc}")
            vl = kvp.tile([128, D1], F16, tag=f"vl{kc}")
            ps = pp.tile([128, 512], F32, tag="ps")
            nc.tensor.matmul(ps[:], ones_sb[:], b1v_sb[0][:], start=True, stop=False)
            nc.tensor.matmul(ps[:], ones_sb[:], b1v_sb[1][:], start=False, stop=False)
            for dc in range(NC1):
                _pair_mms(nc, ps[:],
                          (src_hi[dc][:, kc * 128:(kc + 1) * 128],
                           src_lo[dc][:, kc * 128:(kc + 1) * 128]),
                          (w1sb["v"][0][dc][:], w1sb["v"][1][dc][:]),
                          start=False)
            nc.vector.tensor_copy(vh[:], ps[:])
            nc.vector.tensor_tensor(vl[:], ps[:], vh[:], op=ALU.subtract)
            v_hi.append(vh)
            v_lo.append(vl)

        for qi in range(NQ1):
            qsl = slice(qi * QT, (qi + 1) * QT)
            # Q^T for this tile: psum [128, 4*128], chunk ec at cols ec*128
            qps = pp.tile([128, 512], F32, tag="ps")
            for ec in range(NC1):
                for dc in range(NC1):
                    _pair_mms(nc, qps[:, ec * 128:(ec + 1) * 128],
                              (w1sb["q"][0][dc][:, ec * 128:(ec + 1) * 128],
                               w1sb["q"][1][dc][:, ec * 128:(ec + 1) * 128]),
                              (src_hi[dc][:, qsl], src_lo[dc][:, qsl]),
                              start=(dc == 0))
            qf = wkp.tile([128, 512], F32, tag="qevac")
            for ec in range(NC1):
                esl = slice(ec * 128, (ec + 1) * 128)
                nc.vector.tensor_scalar(qf[:, esl], qps[:, esl],
                                        b1q_sb[:, ec:ec + 1], None, op0=ALU.add)
            q_hi = wkp.tile([128, 512], F16, tag="qhi")
            q_lo = wkp.tile([128, 512], F16, tag="qlo")
            nc.vector.tensor_copy(q_hi[:], qf[:])
            nc.vector.tensor_tensor(q_lo[:], qf[:], q_hi[:], op=ALU.subtract)

            sps_h = [scp.tile([128, S // 2], F32, name=f"scr{h}", tag="scoresh")
                     for h in range(2)]
            for sc in range(NSC):
                ssl = slice(sc * 512, (sc + 1) * 512)
                hsl = slice((sc % 2) * 512, (sc % 2) * 512 + 512)
                for ec in range(NC1):
                    esl = slice(ec * 128, (ec + 1) * 128)
                    _pair_mms(nc, sps_h[sc // 2][:, hsl],
                              (q_hi[:, esl], q_lo[:, esl]),
                              (kt_hi[ec][:, ssl], kt_lo[ec][:, ssl]),
                              start=(ec == 0))

            pth, ptl, c = _softmax_ptiles(nc, ptp1, ptp2, wkp, sps_h, "1", pair=True)

            ops_h = []
            for h in range(2):
                ops = pp.tile([128, 512], F32, name=f"av{h}", tag="ps")
                for kc8 in range(NKC // 2):
                    kc = h * (NKC // 2) + kc8
                    nc.tensor.matmul(ops[:], pth[h][:, kc8, :], v_hi[kc][:],
                                     start=(kc8 == 0), stop=False)
                    nc.tensor.matmul(ops[:], pth[h][:, kc8, :], v_lo[kc][:],
                                     start=False, stop=False)
                    nc.tensor.matmul(ops[:], ptl[h][:, kc8, :], v_hi[kc][:],
                                     start=False, stop=(kc8 == NKC // 2 - 1))
                ops_h.append(ops)

            af = ptp1.tile([128, 512], F32, tag="af")
            nc.vector.tensor_scalar(af[:], ops_h[0][:], c[0][:, 0:1], None, op0=ALU.mult)
            af2 = ptp1.tile([128, 512], F32, tag="af2")
            nc.vector.tensor_scalar(af2[:], ops_h[1][:], c[1][:, 0:1], None, op0=ALU.mult)
            nc.vector.tensor_tensor(af[:], af[:], af2[:], op=ALU.add)
            a_hi = wkp.tile([128, 512], F16, tag="ahi")
            a_lo = wkp.tile([128, 512], F16, tag="alo")
            nc.scalar.copy(a_hi[:], af[:])
            nc.vector.tensor_tensor(a_lo[:], af[:], a_hi[:], op=ALU.subtract)
            at_hi = wkp.tile([128, NC1, 128], F16, tag="athi")
            at_lo = wkp.tile([128, NC1, 128], F16, tag="atlo")
            nc.sync.dma_start_transpose(at_hi[:], a_hi[:])
            nc.sync.dma_start_transpose(at_lo[:], a_lo[:])

            # residual in transposed space, then resplit; single strided store
            x1h = wkp.tile([128, NC1, 128], F16, tag="x1h")
            x1l = wkp.tile([128, NC1, 128], F16, tag="x1l")
            for ec in range(NC1):
                r1 = wkp.tile([128, 128], F32, tag="r1")
                nc.vector.tensor_scalar(r1[:], resid_hi[ec][:, qsl],
                                        wres_sb[:, wcol:wcol + 1], None, op0=ALU.mult)
                nc.vector.tensor_tensor(r1[:], r1[:], at_hi[:, ec, :], op=ALU.add)
                r2 = wkp.tile([128, 128], F32, tag="r2")
                nc.vector.tensor_scalar(r2[:], resid_lo[ec][:, qsl],
                                        wres_sb[:, wcol:wcol + 1], None, op0=ALU.mult)
                nc.vector.tensor_tensor(r2[:], r2[:], at_lo[:, ec, :], op=ALU.add)
                nc.vector.tensor_tensor(r1[:], r1[:], r2[:], op=ALU.add)
                nc.scalar.copy(x1h[:, ec, :], r1[:])
                nc.vector.tensor_tensor(x1l[:, ec, :], r1[:], x1h[:, ec, :], op=ALU.subtract)
            oh_ap = o_hi.rearrange("(c p) q -> p c q", p=128)[:, :, qsl]
            ol_ap = o_lo.rearrange("(c p) q -> p c q", p=128)[:, :, qsl]
            nc.gpsimd.dma_start(out=oh_ap, in_=x1h[:])
            nc.gpsimd.dma_start(out=ol_ap, in_=x1l[:])


def _stage2(nc, tc, ttd, w2q, w2k, w2v_hi, b2q_sb, b2k_sb, b2v_sb, ones_sb, out):
    def tt_dram(dc, hi):
        dr = ttd[dc // NC1][0 if hi else 1]
        r = (dc % NC1) * 128
        return dr[r:r + 128, :]

    with (tc.tile_pool(name="s2", bufs=1) as s2p,
          tc.tile_pool(name="s2wk", bufs=2) as wkp,
          tc.tile_pool(name="s2pa", bufs=1) as ptp1,
          tc.tile_pool(name="s2pt", bufs=2) as ptp2,
          tc.tile_pool(name="s2ps", bufs=2, space="PSUM") as pp,
          tc.tile_pool(name="s2sc", bufs=2, space="PSUM") as scp):
        # V2 single fp16 [kc][128, D2]; temp-lo dropped; bias via rank-1
        v2 = []
        with tc.tile_pool(name="w2vp", bufs=1) as wp, \
             tc.tile_pool(name="ttv", bufs=2) as ttp:
            wv = []
            for i in range(NC2):
                t = wp.tile([128, D2], F16, tag=f"w2v{i}")
                nc.gpsimd.dma_start(out=t[:], in_=w2v_hi[i * 128:(i + 1) * 128, :])
                wv.append(t)
            for kcg in range(NKC // 4):
                gsl = slice(kcg * 512, (kcg + 1) * 512)
                tchunks = []
                for dc in range(NC2):
                    t = ttp.tile([128, 512], F16, tag=f"ttv{dc}")
                    nc.gpsimd.dma_start(out=t[:], in_=tt_dram(dc, True)[:, gsl])
                    tchunks.append(t)
                for kcl in range(4):
                    kc = kcg * 4 + kcl
                    lsl = slice(kcl * 128, (kcl + 1) * 128)
                    vt = s2p.tile([128, D2], F16, name=f"v2_{kc}", tag=f"v2{kc}")
                    for e2c in range(2):
                        esl = slice(e2c * 512, (e2c + 1) * 512)
                        ps = pp.tile([128, 512], F32, tag="ps2")
                        nc.tensor.matmul(ps[:], ones_sb[:], b2v_sb[0][:, esl],
                                         start=True, stop=False)
                        nc.tensor.matmul(ps[:], ones_sb[:], b2v_sb[1][:, esl],
                                         start=False, stop=False)
                        for dc in range(NC2):
                            nc.tensor.matmul(ps[:], tchunks[dc][:, lsl], wv[dc][:, esl],
                                             start=False, stop=(dc == NC2 - 1))
                        nc.vector.tensor_copy(vt[:, esl], ps[:])
                    v2.append(vt)

        # K2^T pair [ec][128, S]; tempT pair streamed by s-chunk
        k2_hi = [s2p.tile([128, S], F16, name=f"k2h{ec}", tag=f"k2h{ec}") for ec in range(NC2)]
        k2_lo = [s2p.tile([128, S], F16, name=f"k2l{ec}", tag=f"k2l{ec}") for ec in range(NC2)]
        with tc.tile_pool(name="w2ks", bufs=1) as wks, \
             tc.tile_pool(name="ttk", bufs=1) as ttp:
            for sc in range(NSC):
                ssl = slice(sc * 512, (sc + 1) * 512)
                tch, tcl = [], []
                for dc in range(NC2):
                    th = ttp.tile([128, 512], F16, tag=f"ttkh{dc}")
                    tl = ttp.tile([128, 512], F16, tag=f"ttkl{dc}")
                    nc.gpsimd.dma_start(out=th[:], in_=tt_dram(dc, True)[:, ssl])
                    nc.gpsimd.dma_start(out=tl[:], in_=tt_dram(dc, False)[:, ssl])
                    tch.append(th)
                    tcl.append(tl)
                for e2h in range(2):
                    wsl = slice(e2h * 512, (e2h + 1) * 512)
                    wrh, wrl = [], []
                    for dc in range(NC2):
                        wh = wks.tile([128, 512], F16, name=f"wkh{dc}", tag=f"wkh{dc}")
                        wl = wks.tile([128, 512], F16, name=f"wkl{dc}", tag=f"wkl{dc}")
                        nc.gpsimd.dma_start(out=wh[:], in_=w2k[0][dc * 128:(dc + 1) * 128, wsl])
                        nc.gpsimd.dma_start(out=wl[:], in_=w2k[1][dc * 128:(dc + 1) * 128, wsl])
                        wrh.append(wh)
                        wrl.append(wl)
                    for ecl in range(4):
                        ec = e2h * 4 + ecl
                        lsl = slice(ecl * 128, (ecl + 1) * 128)
                        ps = pp.tile([128, 512], F32, tag="ps2")
                        for dc in range(NC2):
                            _pair_mms(nc, ps[:],
                                      (wrh[dc][:, lsl], wrl[dc][:, lsl]),
                                      (tch[dc][:], tcl[dc][:]),
                                      start=(dc == 0))
                        kf = wkp.tile([128, 512], F32, tag="k2evac")
                        nc.vector.tensor_scalar(kf[:], ps[:], b2k_sb[:, ec:ec + 1], None,
                                                op0=ALU.add)
                        nc.vector.tensor_copy(k2_hi[ec][:, ssl], kf[:])
                        nc.vector.tensor_tensor(k2_lo[ec][:, ssl], kf[:], k2_hi[ec][:, ssl],
                                                op=ALU.subtract)

        # Q2^T pair for device rows [0:SH)
        q2_hi = [s2p.tile([128, SH], F16, name=f"q2h{ec}", tag=f"q2h{ec}") for ec in range(NC2)]
        q2_lo = [s2p.tile([128, SH], F16, name=f"q2l{ec}", tag=f"q2l{ec}") for ec in range(NC2)]
        with tc.tile_pool(name="w2qs", bufs=1) as wqs, \
             tc.tile_pool(name="ttq", bufs=1) as ttp:
            for sc in range(SH // 512):
                ssl = slice(sc * 512, (sc + 1) * 512)
                tch, tcl = [], []
                for dc in range(NC2):
                    th = ttp.tile([128, 512], F16, tag=f"ttqh{dc}")
                    tl = ttp.tile([128, 512], F16, tag=f"ttql{dc}")
                    nc.gpsimd.dma_start(out=th[:], in_=tt_dram(dc, True)[:, ssl])
                    nc.gpsimd.dma_start(out=tl[:], in_=tt_dram(dc, False)[:, ssl])
                    tch.append(th)
                    tcl.append(tl)
                for e2h in range(2):
                    wsl = slice(e2h * 512, (e2h + 1) * 512)
                    wrh, wrl = [], []
                    for dc in range(NC2):
                        wh = wqs.tile([128, 512], F16, name=f"wqh{dc}", tag=f"wqh{dc}")
                        wl = wqs.tile([128, 512], F16, name=f"wql{dc}", tag=f"wql{dc}")
                        nc.gpsimd.dma_start(out=wh[:], in_=w2q[0][dc * 128:(dc + 1) * 128, wsl])
                        nc.gpsimd.dma_start(out=wl[:], in_=w2q[1][dc * 128:(dc + 1) * 128, wsl])
                        wrh.append(wh)
                        wrl.append(wl)
                    for ecl in range(4):
                        ec = e2h * 4 + ecl
                        lsl = slice(ecl * 128, (ecl + 1) * 128)
                        ps = pp.tile([128, 512], F32, tag="ps2")
                        for dc in range(NC2):
                            _pair_mms(nc, ps[:],
                                      (wrh[dc][:, lsl], wrl[dc][:, lsl]),
                                      (tch[dc][:], tcl[dc][:]),
                                      start=(dc == 0))
                        qf = wkp.tile([128, 512], F32, tag="q2evac")
                        nc.vector.tensor_scalar(qf[:], ps[:], b2q_sb[:, ec:ec + 1], None,
                                                op0=ALU.add)
                        nc.vector.tensor_copy(q2_hi[ec][:, ssl], qf[:])
                        nc.vector.tensor_tensor(q2_lo[ec][:, ssl], qf[:], q2_hi[ec][:, ssl],
                                                op=ALU.subtract)

        # attention over my 8 q-tiles
        for qi in range(NQ2):
            qsl = slice(qi * QT, (qi + 1) * QT)
            sps_h = [scp.tile([128, S // 2], F32, name=f"s2scr{h}", tag="s2scoresh")
                     for h in range(2)]
            for sc in range(NSC):
                ssl = slice(sc * 512, (sc + 1) * 512)
                hsl = slice((sc % 2) * 512, (sc % 2) * 512 + 512)
                for ec in range(NC2):
                    _pair_mms(nc, sps_h[sc // 2][:, hsl],
                              (q2_hi[ec][:, qsl], q2_lo[ec][:, qsl]),
                              (k2_hi[ec][:, ssl], k2_lo[ec][:, ssl]),
                              start=(ec == 0))

            pth, _, c = _softmax_ptiles(nc, ptp1, ptp2, wkp, sps_h, "2", pair=False)

            ops_h = []
            for h in range(2):
                ops = pp.tile([128, D2], F32, name=f"av2{h}", tag="ps2")
                for e2c in range(2):
                    esl = slice(e2c * 512, (e2c + 1) * 512)
                    for kc8 in range(NKC // 2):
                        kc = h * (NKC // 2) + kc8
                        nc.tensor.matmul(ops[:, esl], pth[h][:, kc8, :], v2[kc][:, esl],
                                         start=(kc8 == 0), stop=(kc8 == NKC // 2 - 1))
                ops_h.append(ops)
            of = ptp1.tile([128, D2], F32, tag="of2")
            nc.vector.tensor_scalar(of[:], ops_h[0][:], c[0][:, 0:1], None, op0=ALU.mult)
            of2 = ptp1.tile([128, D2], F32, tag="of2b")
            nc.vector.tensor_scalar(of2[:], ops_h[1][:], c[1][:, 0:1], None, op0=ALU.mult)
            nc.vector.tensor_tensor(of[:], of[:], of2[:], op=ALU.add)
            nc.sync.dma_start(out=out[qsl, :], in_=of[:])


def _prep_inputs(inputs):
    x = np.asarray(inputs["x"], np.float32)
    y = np.asarray(inputs["y"], np.float32)
    w1v = float(np.asarray(inputs["weight1"]).reshape(-1)[0])
    w2v = float(np.asarray(inputs["weight2"]).reshape(-1)[0])

    shared = {}
    for t in "qkv":
        wt = np.ascontiguousarray(np.asarray(inputs[f"sa1_W{t}"], np.float32).T)
        shared[f"w1{t}_hi"], shared[f"w1{t}_lo"] = _split16(wt)
    for t in "qk":
        wt = np.ascontiguousarray(np.asarray(inputs[f"sa2_W{t}"], np.float32).T)
        shared[f"w2{t}_hi"], shared[f"w2{t}_lo"] = _split16(wt)
    shared["w2v_hi"] = np.ascontiguousarray(
        np.asarray(inputs["sa2_Wv"], np.float32).T).astype(np.float16)

    shared["b1q"] = np.ascontiguousarray(
        np.asarray(inputs["sa1_bq"], np.float32).reshape(NC1, 128).T)
    shared["b1k"] = np.ascontiguousarray(
        np.asarray(inputs["sa1_bk"], np.float32).reshape(NC1, 128).T)
    shared["b2q"] = np.ascontiguousarray(
        np.asarray(inputs["sa2_bq"], np.float32).reshape(NC2, 128).T)
    shared["b2k"] = np.ascontiguousarray(
        np.asarray(inputs["sa2_bk"], np.float32).reshape(NC2, 128).T)
    shared["b1v_hi"], shared["b1v_lo"] = _split16(
        np.asarray(inputs["sa1_bv"], np.float32).reshape(1, D1))
    shared["b2v_hi"], shared["b2v_lo"] = _split16(
        np.asarray(inputs["sa2_bv"], np.float32).reshape(1, D2))
    shared["ones1"] = np.ones((1, 128), np.float16)
    shared["wres"] = np.broadcast_to(
        np.array([[w2v, w1v]], np.float32), (128, 2)).copy()

    in_maps = []
    for c in range(8):
        b, h = c // 2, c % 2
        m = dict(shared)
        for name, arr in [("x", x[b]), ("y", y[b])]:
            rolled = np.roll(arr, -h * SH, axis=0) if h else arr
            tr = np.ascontiguousarray(rolled.T)
            m[f"{name}t_hi"], m[f"{name}t_lo"] = _split16(tr)
        in_maps.append(m)
    return in_maps


def kernel(**inputs):
    if "nc" not in _CACHED:
        _CACHED["nc"] = _build()
    nc = _CACHED["nc"]
    in_maps = _prep_inputs(inputs)
    import time as _time
    _t0 = _time.time()
    res = run_bass_kernel_spmd(nc, in_maps, list(range(8)))
    _CACHED["exec_wall"] = _time.time() - _t0
    _CACHED["last_res"] = res
    out = np.empty((B, S, D2), np.float32)
    for c in range(8):
        b, h = c // 2, c % 2
        out[b, h * SH:(h + 1) * SH, :] = res.results[c]["out"]
    return out



# revision 7
# speedup vs baseline: 2.4193x; 2.4193x over previous
"""Trainium2 Bass kernel for nn_Cross_attention_dl_91061896610498.

Wall-clock through the axon tunnel is dominated by host->device bytes, so
each core uploads only unique data: its query-half of x/y (fp32,
pre-transposed) plus 1/8 of a packed weight payload.  On-device AllGathers
rebuild the full tensors (pair groups for x/y and the stage-1 -> stage-2
temp, 8-way for weights).  fp16 hi/lo splits are computed on device; the
three matmul pair-products per fp32 matmul keep Q/K/score accuracy (no
1/sqrt(d) scaling -> near-one-hot softmax).  Stage 1 is pair-split (each
core computes its query-half of x1/y1), stage 2 runs on the core's half.
Output returns as fp16.

Core c = (batch b=c//2, half h=c%2).  Core uploads:
  xq/yq  [512,1024] f32  - x[b].T columns [h*1024:(h+1)*1024]
  wp32   [353,1024] f32  - 1/8 slice of fp32 payload (W1qkv^T, W2qk^T, biases)
  wp16   [128,1024] f16  - 1/8 slice of W2v^T fp16
"""

import numpy as np

import concourse.bass as bass
import concourse.mybir as mybir
from concourse.tile import TileContext
from concourse.bass_utils import run_bass_kernel_spmd

F16 = mybir.dt.float16
F32 = mybir.dt.float32
AF = mybir.ActivationFunctionType
ALU = mybir.AluOpType
AX = mybir.AxisListType

D1, D2, B, S = 512, 1024, 4, 2048
SH = S // 2          # per-core query half
QT = 128             # query tile
NQ = SH // QT        # q tiles per core (8, both stages)
NC1 = D1 // 128      # 4 partition chunks of D1
NC2 = D2 // 128      # 8 partition chunks of D2
NKC = S // 128       # 16 key chunks
NSC = S // 512       # 4 moving chunks over S

# fp32 payload layout (rows of 1024 f32)
OW1Q, OW1K, OW1V = 0, 256, 512          # [512,512] each, flat as [256,1024]
OW2Q, OW2K = 768, 1792                  # [1024,1024] each
OB1Q, OB1K, OB1V = 2816, 2817, 2818    # bias rows
OB2Q, OB2K, OB2V, OWRES = 2819, 2820, 2821, 2822
PR32 = 2824                             # padded to /8
PC32 = PR32 // 8                        # 353 rows per core
PR16 = 1024                             # w2v^T rows
PC16 = PR16 // 8                        # 128 rows per core

PAIRS = [[0, 1], [2, 3], [4, 5], [6, 7]]
FULL = [list(range(8))]

_CACHED = {}


def _fix_excess_waits(nc, max_waits=1):
    """walrus in this env accepts only 1 sync-wait per instruction; move
    excess waits onto preceding same-engine NOPs."""
    ctr = 0
    for fn in nc.m.functions:
        for blk in fn.blocks:
            insts = blk.bb.instructions if hasattr(blk, "bb") else blk.instructions
            new = []
            changed = False
            for inst in insts:
                si = inst.sync_info
                waits = list(si.on_wait) if (si is not None and si.on_wait) else []
                if len(waits) > max_waits:
                    excess, keep = waits[:-max_waits], waits[-max_waits:]
                    while excess:
                        chunk, excess = excess[:max_waits], excess[max_waits:]
                        ctr += 1
                        nop = mybir.InstNoOp(name=f"I-waitfix-{ctr}", engine=inst.engine)
                        nop.sync_info = mybir.SyncInfo(on_wait=chunk, on_update=[])
                        new.append(nop)
                    inst.sync_info = mybir.SyncInfo(
                        on_wait=keep,
                        on_update=list(si.on_update) if si.on_update else [],
                    )
                    changed = True
                new.append(inst)
            if changed:
                if hasattr(blk, "bb"):
                    blk.bb.instructions = new
                else:
                    blk.instructions = new
    return ctr


def _pair_mms(nc, psum, lhs_pair, rhs_pair, start, stop=False):
    """Accumulate (lhs_hi+lhs_lo).T @ (rhs_hi+rhs_lo) into psum (lo*lo dropped)."""
    lh, ll = lhs_pair
    rh, rl = rhs_pair
    nc.tensor.matmul(psum, lh, rh, start=start, stop=False)
    nc.tensor.matmul(psum, lh, rl, start=False, stop=False)
    nc.tensor.matmul(psum, ll, rh, start=False, stop=stop)


def _split_rows(nc, pool, wkp, src_ap_fn, nrows, ncols, tag):
    """Load fp32 DRAM rows -> f16 hi/lo SBUF tile pairs [nrows//128][128, ncols].

    src_ap_fn(i) gives the DRAM AP for rows [i*128:(i+1)*128].
    """
    his, los = [], []
    for i in range(nrows // 128):
        st = wkp.tile([128, ncols], F32, tag=f"st{ncols}")
        nc.sync.dma_start(out=st[:], in_=src_ap_fn(i))
        th = pool.tile([128, ncols], F16, tag=f"{tag}_h{i}")
        tl = pool.tile([128, ncols], F16, tag=f"{tag}_l{i}")
        nc.vector.tensor_copy(th[:], st[:])
        nc.vector.tensor_tensor(tl[:], st[:], th[:], op=ALU.subtract)
        his.append(th)
        los.append(tl)
    return his, los


def _softmax_ptiles(nc, pp1, pp2, wkp, sps_h, tag, pair):
    """negmax -> exp (+row sums) -> fp16 (pair) split -> transposed halves."""
    nm = [wkp.tile([128, 1], F32, name=f"nm{tag}{h}", tag=f"nm{tag}{h}") for h in range(2)]
    ls = [wkp.tile([128, 1], F32, name=f"ls{tag}{h}", tag=f"ls{tag}{h}") for h in range(2)]
    pth_halves, ptl_halves = [], []
    for h in range(2):
        nc.vector.reduce_max(nm[h][:], sps_h[h][:], axis=AX.X, negate=True)
        pf = pp1.tile([128, S // 2], F32, tag=f"pf{tag}")
        nc.scalar.activation(pf[:], sps_h[h][:], AF.Exp,
                             bias=nm[h][:, 0:1], accum_out=ls[h][:])
        p_hi = pp1.tile([128, S // 2], F16, tag=f"phi{tag}")
        nc.scalar.copy(p_hi[:], pf[:])
        pth = pp2.tile([128, NKC // 2, 128], F16, tag=f"pth{tag}")
        nc.sync.dma_start_transpose(pth[:], p_hi[:])
        pth_halves.append(pth)
        if pair:
            p_lo = pp1.tile([128, S // 2], F16, tag=f"plo{tag}")
            nc.vector.tensor_tensor(p_lo[:], pf[:], p_hi[:], op=ALU.subtract)
            ptl = pp2.tile([128, NKC // 2, 128], F16, tag=f"ptl{tag}")
            nc.sync.dma_start_transpose(ptl[:], p_lo[:])
            ptl_halves.append(ptl)
    negm = wkp.tile([128, 1], F32, tag=f"negm{tag}")
    nc.vector.tensor_tensor(negm[:], nm[0][:], nm[1][:], op=ALU.min)
    sh = []
    lw = [wkp.tile([128, 1], F32, name=f"lw{tag}{h}", tag=f"lw{tag}{h}") for h in range(2)]
    for h in range(2):
        d = wkp.tile([128, 1], F32, name=f"d{tag}{h}", tag=f"d{tag}{h}")
        nc.vector.tensor_tensor(d[:], negm[:], nm[h][:], op=ALU.subtract)  # m_h - m <= 0
        s = wkp.tile([128, 1], F32, name=f"sh{tag}{h}", tag=f"sh{tag}{h}")
        nc.scalar.activation(s[:], d[:], AF.Exp)
        sh.append(s)
        nc.vector.tensor_tensor(lw[h][:], ls[h][:], s[:], op=ALU.mult)
    lsum = wkp.tile([128, 1], F32, tag=f"lsum{tag}")
    nc.vector.tensor_tensor(lsum[:], lw[0][:], lw[1][:], op=ALU.add)
    rl = wkp.tile([128, 1], F32, tag=f"rl{tag}")
    nc.vector.reciprocal(rl[:], lsum[:])
    c = []
    for h in range(2):
        ch = wkp.tile([128, 1], F32, name=f"c{tag}{h}", tag=f"c{tag}{h}")
        nc.vector.tensor_tensor(ch[:], sh[h][:], rl[:], op=ALU.mult)
        c.append(ch)
    return pth_halves, ptl_halves, c


def _build():
    import concourse.tile_utils as tile_utils
    tile_utils.max_sbuf_usage = 204 * 1024

    nc = bass.Bass("TRN2", target_bir_lowering=False, debug=False)

    xq = nc.dram_tensor("xq", [D1, SH], F32, kind="ExternalInput")
    yq = nc.dram_tensor("yq", [D1, SH], F32, kind="ExternalInput")
    wp32 = nc.dram_tensor("wp32", [PC32, 1024], F32, kind="ExternalInput")
    wp16 = nc.dram_tensor("wp16", [PC16, 1024], F16, kind="ExternalInput")
    out = nc.dram_tensor("out", [SH, D2], F16, kind="ExternalOutput")

    # collective bounce buffers (collectives can't touch External tensors)
    xb = nc.dram_tensor("xb", [D1, SH], F32)
    yb = nc.dram_tensor("yb", [D1, SH], F32)
    w32b = nc.dram_tensor("w32b", [PC32, 1024], F32)
    w16b = nc.dram_tensor("w16b", [PC16, 1024], F16)
    # gathered: xg/yg rows [h*512:(h+1)*512] = x^T cols [h*1024:(h+1)*1024]
    xg = nc.dram_tensor("xg", [2 * D1, SH], F32)
    yg = nc.dram_tensor("yg", [2 * D1, SH], F32)
    w32g = nc.dram_tensor("w32g", [PR32, 1024], F32, addr_space="Shared")
    w16g = nc.dram_tensor("w16g", [PR16, 1024], F16, addr_space="Shared")

    # my tempT half (stage-1 outputs, transposed: [D1, my 1024 q cols])
    x1h_d = nc.dram_tensor("x1h_d", [D1, SH], F16)
    x1l_d = nc.dram_tensor("x1l_d", [D1, SH], F16)
    y1h_d = nc.dram_tensor("y1h_d", [D1, SH], F16)
    y1l_d = nc.dram_tensor("y1l_d", [D1, SH], F16)
    # pair-gathered tempT: rows [h*512:(h+1)*512] = cols [h*1024:(h+1)*1024]
    tgxh = nc.dram_tensor("tgxh", [2 * D1, SH], F16)
    tgxl = nc.dram_tensor("tgxl", [2 * D1, SH], F16)
    tgyh = nc.dram_tensor("tgyh", [2 * D1, SH], F16)
    tgyl = nc.dram_tensor("tgyl", [2 * D1, SH], F16)

    def cc(groups, i, o):
        nc.gpsimd.collective_compute(
            "AllGather", ALU.bypass, replica_groups=groups,
            ins=[i[:].opt()], outs=[o[:].opt()],
        )

    with TileContext(nc) as tc:
        for b_, s_ in [(xb, xq), (yb, yq), (w32b, wp32), (w16b, wp16)]:
            nc.gpsimd.dma_start(out=b_[:], in_=s_[:])
        cc(PAIRS, xb, xg)
        cc(PAIRS, yb, yg)
        cc(FULL, w32b, w32g)
        cc(FULL, w16b, w16g)

        # payload views
        v512 = w32g.rearrange("r (k c) -> (r k) c", k=2)       # [5648, 512]
        vp8 = w32g.rearrange("r (p j) -> p r j", p=128)        # [128, 2824, 8]

        with tc.tile_pool(name="const", bufs=1) as cp:
            b1q_sb = cp.tile([128, 1, 8], F32, tag="b1q")
            b1k_sb = cp.tile([128, 1, 8], F32, tag="b1k")
            b2q_sb = cp.tile([128, 1, 8], F32, tag="b2q")
            b2k_sb = cp.tile([128, 1, 8], F32, tag="b2k")
            wres_sb = cp.tile([128, 1, 8], F32, tag="wres")
            for sb, row in [(b1q_sb, OB1Q), (b1k_sb, OB1K), (b2q_sb, OB2Q),
                            (b2k_sb, OB2K), (wres_sb, OWRES)]:
                nc.sync.dma_start(out=sb[:], in_=vp8[:, row:row + 1, :])
            b1v_sb = (cp.tile([1, D1], F16, name="b1vh", tag="b1vh"),
                      cp.tile([1, D1], F16, name="b1vl", tag="b1vl"))
            b2v_sb = (cp.tile([1, D2], F16, name="b2vh", tag="b2vh"),
                      cp.tile([1, D2], F16, name="b2vl", tag="b2vl"))
            with tc.tile_pool(name="cwk", bufs=1) as cwk:
                b1v_f = cwk.tile([1, D1], F32, tag="b1vf")
                b2v_f = cwk.tile([1, D2], F32, tag="b2vf")
                nc.sync.dma_start(out=b1v_f[:], in_=w32g[OB1V:OB1V + 1, 0:D1])
                nc.sync.dma_start(out=b2v_f[:], in_=w32g[OB2V:OB2V + 1, :])
                for (th, tl), tf in [(b1v_sb, b1v_f), (b2v_sb, b2v_f)]:
                    nc.vector.tensor_copy(th[:], tf[:])
                    nc.vector.tensor_tensor(tl[:], tf[:], th[:], op=ALU.subtract)
            ones_sb = cp.tile([1, 128], F16, tag="ones1")
            nc.vector.memset(ones_sb[:], 1.0)

            # ---------------- stage 1 ----------------
            with tc.tile_pool(name="loc", bufs=1) as locp:
                with tc.tile_pool(name="lwk", bufs=2) as lwk:
                    # W1^T chunk pairs [t][dc][128, 512]
                    w1sb = {}
                    for t, off in [("q", OW1Q), ("k", OW1K), ("v", OW1V)]:
                        w1sb[t] = _split_rows(
                            nc, locp, lwk,
                            lambda i, o=off: v512[2 * o + i * 128:2 * o + (i + 1) * 128, :],
                            D1, D1, f"w1{t}")
                    # local q-half pairs [dc][128, 1024]
                    xql = _split_rows(nc, locp, lwk,
                                      lambda i: xq[i * 128:(i + 1) * 128, :],
                                      D1, SH, "xql")
                    yql = _split_rows(nc, locp, lwk,
                                      lambda i: yq[i * 128:(i + 1) * 128, :],
                                      D1, SH, "yql")

                for ti, (src_g, q_loc, r_loc, wcol, o_hi, o_lo) in enumerate([
                        (xg, xql, yql, 0, x1h_d, x1l_d),
                        (yg, yql, xql, 1, y1h_d, y1l_d)]):
                    _stage1_attn(nc, tc, ti, src_g, q_loc, r_loc, wcol, o_hi, o_lo,
                                 w1sb, b1q_sb, b1k_sb, b1v_sb, ones_sb, wres_sb)

            cc(PAIRS, x1h_d, tgxh)
            cc(PAIRS, y1h_d, tgyh)
            cc(PAIRS, x1l_d, tgxl)
            cc(PAIRS, y1l_d, tgyl)

            # ---------------- stage 2 ----------------
            _stage2(nc, tc, (tgxh, tgxl, tgyh, tgyl),
                    (x1h_d, x1l_d, y1h_d, y1l_d), w32g, w16g,
                    b2q_sb, b2k_sb, b2v_sb, ones_sb, out)

    _fix_excess_waits(nc)
    return nc


def _stage1_attn(nc, tc, ti, src_g, q_loc, r_loc, wcol, o_hi, o_lo,
                 w1sb, b1q_sb, b1k_sb, b1v_sb, ones_sb, wres_sb):
    q_hi_loc, q_lo_loc = q_loc
    r_hi_loc, r_lo_loc = r_loc
    with (tc.tile_pool(name=f"kv{ti}", bufs=1) as kvp,
          tc.tile_pool(name=f"wk{ti}", bufs=2) as wkp,
          tc.tile_pool(name=f"ps{ti}", bufs=4, space="PSUM") as pp,
          tc.tile_pool(name=f"sc{ti}", bufs=2, space="PSUM") as scp):
        kt_hi = [kvp.tile([128, S], F16, name=f"kth{ec}", tag=f"kth{ec}")
                 for ec in range(NC1)]
        kt_lo = [kvp.tile([128, S], F16, name=f"ktl{ec}", tag=f"ktl{ec}")
                 for ec in range(NC1)]
        v_hi = [kvp.tile([128, D1], F16, name=f"vh{kc}", tag=f"vh{kc}")
                for kc in range(NKC)]
        v_lo = [kvp.tile([128, D1], F16, name=f"vl{kc}", tag=f"vl{kc}")
                for kc in range(NKC)]

        # full-sequence source pairs (scoped: freed before the q loop)
        with tc.tile_pool(name=f"src{ti}", bufs=1) as srcp, \
             tc.tile_pool(name=f"swk{ti}", bufs=2) as swk:
            src_hi, src_lo = [], []
            for dc in range(NC1):
                th = srcp.tile([128, S], F16, name=f"sfh{dc}", tag=f"sfh{dc}")
                tl = srcp.tile([128, S], F16, name=f"sfl{dc}", tag=f"sfl{dc}")
                for hh in range(2):
                    st = swk.tile([128, SH], F32, tag="st_src")
                    nc.sync.dma_start(
                        out=st[:],
                        in_=src_g[hh * D1 + dc * 128:hh * D1 + (dc + 1) * 128, :])
                    csl = slice(hh * SH, (hh + 1) * SH)
                    nc.vector.tensor_copy(th[:, csl], st[:])
                    nc.vector.tensor_tensor(tl[:, csl], st[:], th[:, csl],
                                            op=ALU.subtract)
                src_hi.append(th)
                src_lo.append(tl)

            # K^T pair [ec][128, S]
            for ec in range(NC1):
                for sc in range(NSC):
                    ssl = slice(sc * 512, (sc + 1) * 512)
                    ps = pp.tile([128, 512], F32, tag="ps")
                    for dc in range(NC1):
                        _pair_mms(nc, ps[:],
                                  (w1sb["k"][0][dc][:, ec * 128:(ec + 1) * 128],
                                   w1sb["k"][1][dc][:, ec * 128:(ec + 1) * 128]),
                                  (src_hi[dc][:, ssl], src_lo[dc][:, ssl]),
                                  start=(dc == 0))
                    kf = wkp.tile([128, 512], F32, tag="kevac")
                    nc.vector.tensor_scalar(kf[:], ps[:], b1k_sb[:, 0, ec:ec + 1],
                                            None, op0=ALU.add)
                    nc.vector.tensor_copy(kt_hi[ec][:, ssl], kf[:])
                    nc.vector.tensor_tensor(kt_lo[ec][:, ssl], kf[:], kt_hi[ec][:, ssl],
                                            op=ALU.subtract)

            # V pair [kc][128, D1]; bias via rank-1 ones x b1v
            for kc in range(NKC):
                ps = pp.tile([128, 512], F32, tag="ps")
                nc.tensor.matmul(ps[:], ones_sb[:], b1v_sb[0][:], start=True, stop=False)
                nc.tensor.matmul(ps[:], ones_sb[:], b1v_sb[1][:], start=False, stop=False)
                for dc in range(NC1):
                    _pair_mms(nc, ps[:],
                              (src_hi[dc][:, kc * 128:(kc + 1) * 128],
                               src_lo[dc][:, kc * 128:(kc + 1) * 128]),
                              (w1sb["v"][0][dc][:], w1sb["v"][1][dc][:]),
                              start=False)
                nc.vector.tensor_copy(v_hi[kc][:], ps[:])
                nc.vector.tensor_tensor(v_lo[kc][:], ps[:], v_hi[kc][:], op=ALU.subtract)

        with (tc.tile_pool(name=f"pa{ti}", bufs=1) as ptp1,
              tc.tile_pool(name=f"pt{ti}", bufs=2) as ptp2):
            _stage1_qloop(nc, qi_pool=(ptp1, ptp2, pp, scp, wkp), w1sb=w1sb,
                          q_loc=(q_hi_loc, q_lo_loc), r_loc=(r_hi_loc, r_lo_loc),
                          kt=(kt_hi, kt_lo), v=(v_hi, v_lo), wcol=wcol,
                          b1q_sb=b1q_sb, wres_sb=wres_sb, o_hi=o_hi, o_lo=o_lo)


def _stage1_qloop(nc, qi_pool, w1sb, q_loc, r_loc, kt, v, wcol,
                  b1q_sb, wres_sb, o_hi, o_lo):
    ptp1, ptp2, pp, scp, wkp = qi_pool
    q_hi_loc, q_lo_loc = q_loc
    r_hi_loc, r_lo_loc = r_loc
    kt_hi, kt_lo = kt
    v_hi, v_lo = v
    if True:
        for qi in range(NQ):
            qsl = slice(qi * QT, (qi + 1) * QT)
            # Q^T for this tile from LOCAL half: psum [128, 4*128]
            qps = pp.tile([128, 512], F32, tag="ps")
            for ec in range(NC1):
                for dc in range(NC1):
                    _pair_mms(nc, qps[:, ec * 128:(ec + 1) * 128],
                              (w1sb["q"][0][dc][:, ec * 128:(ec + 1) * 128],
                               w1sb["q"][1][dc][:, ec * 128:(ec + 1) * 128]),
                              (q_hi_loc[dc][:, qsl], q_lo_loc[dc][:, qsl]),
                              start=(dc == 0))
            qf = wkp.tile([128, 512], F32, tag="qevac")
            for ec in range(NC1):
                esl = slice(ec * 128, (ec + 1) * 128)
                nc.vector.tensor_scalar(qf[:, esl], qps[:, esl],
                                        b1q_sb[:, 0, ec:ec + 1], None, op0=ALU.add)
            q_hi = wkp.tile([128, 512], F16, tag="qhi")
            q_lo = wkp.tile([128, 512], F16, tag="qlo")
            nc.vector.tensor_copy(q_hi[:], qf[:])
            nc.vector.tensor_tensor(q_lo[:], qf[:], q_hi[:], op=ALU.subtract)

            sps_h = [scp.tile([128, S // 2], F32, name=f"scr{h}", tag="scoresh")
                     for h in range(2)]
            for sc in range(NSC):
                ssl = slice(sc * 512, (sc + 1) * 512)
                hsl = slice((sc % 2) * 512, (sc % 2) * 512 + 512)
                for ec in range(NC1):
                    esl = slice(ec * 128, (ec + 1) * 128)
                    _pair_mms(nc, sps_h[sc // 2][:, hsl],
                              (q_hi[:, esl], q_lo[:, esl]),
                              (kt_hi[ec][:, ssl], kt_lo[ec][:, ssl]),
                              start=(ec == 0))

            pth, ptl, c = _softmax_ptiles(nc, ptp1, ptp2, wkp, sps_h, "1", pair=True)

            ops_h = []
            for h in range(2):
                ops = pp.tile([128, 512], F32, name=f"av{h}", tag="ps")
                for kc8 in range(NKC // 2):
                    kc = h * (NKC // 2) + kc8
                    nc.tensor.matmul(ops[:], pth[h][:, kc8, :], v_hi[kc][:],
                                     start=(kc8 == 0), stop=False)
                    nc.tensor.matmul(ops[:], pth[h][:, kc8, :], v_lo[kc][:],
                                     start=False, stop=False)
                    nc.tensor.matmul(ops[:], ptl[h][:, kc8, :], v_hi[kc][:],
                                     start=False, stop=(kc8 == NKC // 2 - 1))
                ops_h.append(ops)

            af = ptp1.tile([128, 512], F32, tag="af")
            nc.vector.tensor_scalar(af[:], ops_h[0][:], c[0][:, 0:1], None, op0=ALU.mult)
            af2 = ptp1.tile([128, 512], F32, tag="af2")
            nc.vector.tensor_scalar(af2[:], ops_h[1][:], c[1][:, 0:1], None, op0=ALU.mult)
            nc.vector.tensor_tensor(af[:], af[:], af2[:], op=ALU.add)
            a_hi = wkp.tile([128, 512], F16, tag="ahi")
            a_lo = wkp.tile([128, 512], F16, tag="alo")
            nc.scalar.copy(a_hi[:], af[:])
            nc.vector.tensor_tensor(a_lo[:], af[:], a_hi[:], op=ALU.subtract)
            at_hi = wkp.tile([128, NC1, 128], F16, tag="athi")
            at_lo = wkp.tile([128, NC1, 128], F16, tag="atlo")
            nc.sync.dma_start_transpose(at_hi[:], a_hi[:])
            nc.sync.dma_start_transpose(at_lo[:], a_lo[:])

            # residual in transposed space, then resplit; single strided store
            x1h = wkp.tile([128, NC1, 128], F16, tag="x1h")
            x1l = wkp.tile([128, NC1, 128], F16, tag="x1l")
            for ec in range(NC1):
                r1 = wkp.tile([128, 128], F32, tag="r1")
                nc.vector.tensor_scalar(r1[:], r_hi_loc[ec][:, qsl],
                                        wres_sb[:, 0, wcol:wcol + 1], None, op0=ALU.mult)
                nc.vector.tensor_tensor(r1[:], r1[:], at_hi[:, ec, :], op=ALU.add)
                r2 = wkp.tile([128, 128], F32, tag="r2")
                nc.vector.tensor_scalar(r2[:], r_lo_loc[ec][:, qsl],
                                        wres_sb[:, 0, wcol:wcol + 1], None, op0=ALU.mult)
                nc.vector.tensor_tensor(r2[:], r2[:], at_lo[:, ec, :], op=ALU.add)
                nc.vector.tensor_tensor(r1[:], r1[:], r2[:], op=ALU.add)
                nc.scalar.copy(x1h[:, ec, :], r1[:])
                nc.vector.tensor_tensor(x1l[:, ec, :], r1[:], x1h[:, ec, :], op=ALU.subtract)
            oh_ap = o_hi.rearrange("(c p) q -> p c q", p=128)[:, :, qsl]
            ol_ap = o_lo.rearrange("(c p) q -> p c q", p=128)[:, :, qsl]
            nc.gpsimd.dma_start(out=oh_ap, in_=x1h[:])
            nc.gpsimd.dma_start(out=ol_ap, in_=x1l[:])


def _stage2(nc, tc, tg, tloc, w32g, w16g, b2q_sb, b2k_sb, b2v_sb, ones_sb, out):
    tgxh, tgxl, tgyh, tgyl = tg
    x1h_d, x1l_d, y1h_d, y1l_d = tloc

    def tt_gath(dc, hi, shalf):
        if dc < NC1:
            dr = tgxh if hi else tgxl
        else:
            dr = tgyh if hi else tgyl
        r = shalf * D1 + (dc % NC1) * 128
        return dr[r:r + 128, :]

    def tt_loc(dc, hi):
        if dc < NC1:
            dr = x1h_d if hi else x1l_d
        else:
            dr = y1h_d if hi else y1l_d
        r = (dc % NC1) * 128
        return dr[r:r + 128, :]

    with (tc.tile_pool(name="s2", bufs=1) as s2p,
          tc.tile_pool(name="s2wk", bufs=2) as wkp,
          tc.tile_pool(name="s2pa", bufs=1) as ptp1,
          tc.tile_pool(name="s2pt", bufs=2) as ptp2,
          tc.tile_pool(name="s2ps", bufs=2, space="PSUM") as pp,
          tc.tile_pool(name="s2sc", bufs=2, space="PSUM") as scp):
        # V2 single fp16 [kc][128, D2]; temp-lo dropped; bias via rank-1
        v2 = []
        with tc.tile_pool(name="w2vp", bufs=1) as wp, \
             tc.tile_pool(name="ttv", bufs=2) as ttp:
            wv = []
            for i in range(NC2):
                t = wp.tile([128, D2], F16, tag=f"w2v{i}")
                nc.gpsimd.dma_start(out=t[:], in_=w16g[i * 128:(i + 1) * 128, :])
                wv.append(t)
            for kcg in range(NKC // 4):
                lsl_g = slice((kcg % 2) * 512, (kcg % 2) * 512 + 512)
                tchunks = []
                for dc in range(NC2):
                    t = ttp.tile([128, 512], F16, tag=f"ttv{dc}")
                    nc.gpsimd.dma_start(out=t[:], in_=tt_gath(dc, True, kcg // 2)[:, lsl_g])
                    tchunks.append(t)
                for kcl in range(4):
                    kc = kcg * 4 + kcl
                    lsl = slice(kcl * 128, (kcl + 1) * 128)
                    vt = s2p.tile([128, D2], F16, name=f"v2_{kc}", tag=f"v2{kc}")
                    for e2c in range(2):
                        esl = slice(e2c * 512, (e2c + 1) * 512)
                        ps = pp.tile([128, 512], F32, tag="ps2")
                        nc.tensor.matmul(ps[:], ones_sb[:], b2v_sb[0][:, esl],
                                         start=True, stop=False)
                        nc.tensor.matmul(ps[:], ones_sb[:], b2v_sb[1][:, esl],
                                         start=False, stop=False)
                        for dc in range(NC2):
                            nc.tensor.matmul(ps[:], tchunks[dc][:, lsl], wv[dc][:, esl],
                                             start=False, stop=(dc == NC2 - 1))
                        nc.vector.tensor_copy(vt[:, esl], ps[:])
                    v2.append(vt)

        # K2^T pair [ec][128, S]; gathered tempT pair + W2k^T f32 streamed
        k2_hi = [s2p.tile([128, S], F16, name=f"k2h{ec}", tag=f"k2h{ec}") for ec in range(NC2)]
        k2_lo = [s2p.tile([128, S], F16, name=f"k2l{ec}", tag=f"k2l{ec}") for ec in range(NC2)]
        with tc.tile_pool(name="w2ks", bufs=1) as wks, \
             tc.tile_pool(name="ttk", bufs=1) as ttp, \
             tc.tile_pool(name="wst", bufs=2) as wst:
            for sc in range(NSC):
                ssl = slice(sc * 512, (sc + 1) * 512)
                lsl_g = slice((sc % 2) * 512, (sc % 2) * 512 + 512)
                tch, tcl = [], []
                for dc in range(NC2):
                    th = ttp.tile([128, 512], F16, tag=f"ttkh{dc}")
                    tl = ttp.tile([128, 512], F16, tag=f"ttkl{dc}")
                    nc.gpsimd.dma_start(out=th[:], in_=tt_gath(dc, True, sc // 2)[:, lsl_g])
                    nc.gpsimd.dma_start(out=tl[:], in_=tt_gath(dc, False, sc // 2)[:, lsl_g])
                    tch.append(th)
                    tcl.append(tl)
                for e2h in range(2):
                    wsl = slice(e2h * 512, (e2h + 1) * 512)
                    wrh, wrl = [], []
                    for dc in range(NC2):
                        st = wst.tile([128, 512], F32, tag="wkst")
                        nc.sync.dma_start(
                            out=st[:],
                            in_=w32g[OW2K + dc * 128:OW2K + (dc + 1) * 128, wsl])
                        wh = wks.tile([128, 512], F16, name=f"wkh{dc}", tag=f"wkh{dc}")
                        wl = wks.tile([128, 512], F16, name=f"wkl{dc}", tag=f"wkl{dc}")
                        nc.vector.tensor_copy(wh[:], st[:])
                        nc.vector.tensor_tensor(wl[:], st[:], wh[:], op=ALU.subtract)
                        wrh.append(wh)
                        wrl.append(wl)
                    for ecl in range(4):
                        ec = e2h * 4 + ecl
                        lsl = slice(ecl * 128, (ecl + 1) * 128)
                        ps = pp.tile([128, 512], F32, tag="ps2")
                        for dc in range(NC2):
                            _pair_mms(nc, ps[:],
                                      (wrh[dc][:, lsl], wrl[dc][:, lsl]),
                                      (tch[dc][:], tcl[dc][:]),
                                      start=(dc == 0))
                        kf = wkp.tile([128, 512], F32, tag="k2evac")
                        nc.vector.tensor_scalar(kf[:], ps[:], b2k_sb[:, 0, ec:ec + 1],
                                                None, op0=ALU.add)
                        nc.vector.tensor_copy(k2_hi[ec][:, ssl], kf[:])
                        nc.vector.tensor_tensor(k2_lo[ec][:, ssl], kf[:], k2_hi[ec][:, ssl],
                                                op=ALU.subtract)

        # Q2^T pair for my SH query columns, from LOCAL tempT
        q2_hi = [s2p.tile([128, SH], F16, name=f"q2h{ec}", tag=f"q2h{ec}") for ec in range(NC2)]
        q2_lo = [s2p.tile([128, SH], F16, name=f"q2l{ec}", tag=f"q2l{ec}") for ec in range(NC2)]
        with tc.tile_pool(name="w2qs", bufs=1) as wqs, \
             tc.tile_pool(name="ttq", bufs=1) as ttp, \
             tc.tile_pool(name="wsq", bufs=2) as wst:
            for sc in range(SH // 512):
                ssl = slice(sc * 512, (sc + 1) * 512)
                tch, tcl = [], []
                for dc in range(NC2):
                    th = ttp.tile([128, 512], F16, tag=f"ttqh{dc}")
                    tl = ttp.tile([128, 512], F16, tag=f"ttql{dc}")
                    nc.gpsimd.dma_start(out=th[:], in_=tt_loc(dc, True)[:, ssl])
                    nc.gpsimd.dma_start(out=tl[:], in_=tt_loc(dc, False)[:, ssl])
                    tch.append(th)
                    tcl.append(tl)
                for e2h in range(2):
                    wsl = slice(e2h * 512, (e2h + 1) * 512)
                    wrh, wrl = [], []
                    for dc in range(NC2):
                        st = wst.tile([128, 512], F32, tag="wqst")
                        nc.sync.dma_start(
                            out=st[:],
                            in_=w32g[OW2Q + dc * 128:OW2Q + (dc + 1) * 128, wsl])
                        wh = wqs.tile([128, 512], F16, name=f"wqh{dc}", tag=f"wqh{dc}")
                        wl = wqs.tile([128, 512], F16, name=f"wql{dc}", tag=f"wql{dc}")
                        nc.vector.tensor_copy(wh[:], st[:])
                        nc.vector.tensor_tensor(wl[:], st[:], wh[:], op=ALU.subtract)
                        wrh.append(wh)
                        wrl.append(wl)
                    for ecl in range(4):
                        ec = e2h * 4 + ecl
                        lsl = slice(ecl * 128, (ecl + 1) * 128)
                        ps = pp.tile([128, 512], F32, tag="ps2")
                        for dc in range(NC2):
                            _pair_mms(nc, ps[:],
                                      (wrh[dc][:, lsl], wrl[dc][:, lsl]),
                                      (tch[dc][:], tcl[dc][:]),
                                      start=(dc == 0))
                        qf = wkp.tile([128, 512], F32, tag="q2evac")
                        nc.vector.tensor_scalar(qf[:], ps[:], b2q_sb[:, 0, ec:ec + 1],
                                                None, op0=ALU.add)
                        nc.vector.tensor_copy(q2_hi[ec][:, ssl], qf[:])
                        nc.vector.tensor_tensor(q2_lo[ec][:, ssl], qf[:], q2_hi[ec][:, ssl],
                                                op=ALU.subtract)

        # attention over my 8 q-tiles
        for qi in range(NQ):
            qsl = slice(qi * QT, (qi + 1) * QT)
            sps_h = [scp.tile([128, S // 2], F32, name=f"s2scr{h}", tag="s2scoresh")
                     for h in range(2)]
            for sc in range(NSC):
                ssl = slice(sc * 512, (sc + 1) * 512)
                hsl = slice((sc % 2) * 512, (sc % 2) * 512 + 512)
                for ec in range(NC2):
                    _pair_mms(nc, sps_h[sc // 2][:, hsl],
                              (q2_hi[ec][:, qsl], q2_lo[ec][:, qsl]),
                              (k2_hi[ec][:, ssl], k2_lo[ec][:, ssl]),
                              start=(ec == 0))

            pth, _, c = _softmax_ptiles(nc, ptp1, ptp2, wkp, sps_h, "2", pair=False)

            ops_h = []
            for h in range(2):
                ops = pp.tile([128, D2], F32, name=f"av2{h}", tag="ps2")
                for e2c in range(2):
                    esl = slice(e2c * 512, (e2c + 1) * 512)
                    for kc8 in range(NKC // 2):
                        kc = h * (NKC // 2) + kc8
                        nc.tensor.matmul(ops[:, esl], pth[h][:, kc8, :], v2[kc][:, esl],
                                         start=(kc8 == 0), stop=(kc8 == NKC // 2 - 1))
                ops_h.append(ops)
            of = ptp1.tile([128, D2], F32, tag="of2")
            nc.vector.tensor_scalar(of[:], ops_h[0][:], c[0][:, 0:1], None, op0=ALU.mult)
            of2 = ptp1.tile([128, D2], F32, tag="of2b")
            nc.vector.tensor_scalar(of2[:], ops_h[1][:], c[1][:, 0:1], None, op0=ALU.mult)
            nc.vector.tensor_tensor(of[:], of[:], of2[:], op=ALU.add)
            o16 = ptp1.tile([128, D2], F16, tag="o16")
            nc.vector.tensor_copy(o16[:], of[:])
            nc.sync.dma_start(out=out[qsl, :], in_=o16[:])


def _row_p8(vec):
    """Pack vec (len n*128, n<=8) into a 1024-row as flat[p*8+j] = vec[j*128+p]."""
    n = len(vec) // 128
    row = np.zeros((128, 8), np.float32)
    row[:, :n] = np.asarray(vec, np.float32).reshape(n, 128).T
    return row.reshape(1024)


def _prep_payload(inputs):
    pay32 = np.zeros((PR32, 1024), np.float32)
    pay32[OW1Q:OW1Q + 256] = np.ascontiguousarray(
        np.asarray(inputs["sa1_Wq"], np.float32).T).reshape(256, 1024)
    pay32[OW1K:OW1K + 256] = np.ascontiguousarray(
        np.asarray(inputs["sa1_Wk"], np.float32).T).reshape(256, 1024)
    pay32[OW1V:OW1V + 256] = np.ascontiguousarray(
        np.asarray(inputs["sa1_Wv"], np.float32).T).reshape(256, 1024)
    pay32[OW2Q:OW2Q + 1024] = np.asarray(inputs["sa2_Wq"], np.float32).T
    pay32[OW2K:OW2K + 1024] = np.asarray(inputs["sa2_Wk"], np.float32).T
    pay32[OB1Q] = _row_p8(inputs["sa1_bq"])
    pay32[OB1K] = _row_p8(inputs["sa1_bk"])
    pay32[OB1V, 0:D1] = np.asarray(inputs["sa1_bv"], np.float32)
    pay32[OB2Q] = _row_p8(inputs["sa2_bq"])
    pay32[OB2K] = _row_p8(inputs["sa2_bk"])
    pay32[OB2V] = np.asarray(inputs["sa2_bv"], np.float32)
    w1v = float(np.asarray(inputs["weight1"]).reshape(-1)[0])
    w2v = float(np.asarray(inputs["weight2"]).reshape(-1)[0])
    wres = np.zeros((128, 8), np.float32)
    wres[:, 0] = w2v
    wres[:, 1] = w1v
    pay32[OWRES] = wres.reshape(1024)
    pay16 = np.ascontiguousarray(
        np.asarray(inputs["sa2_Wv"], np.float32).T).astype(np.float16)
    return pay32, pay16


def _prep_inputs(inputs):
    x = np.asarray(inputs["x"], np.float32)
    y = np.asarray(inputs["y"], np.float32)
    pay32, pay16 = _prep_payload(inputs)
    xt = [np.ascontiguousarray(x[b].T) for b in range(B)]
    yt = [np.ascontiguousarray(y[b].T) for b in range(B)]
    in_maps = []
    for c in range(8):
        b, h = c // 2, c % 2
        csl = slice(h * SH, (h + 1) * SH)
        in_maps.append({
            "xq": xt[b][:, csl],
            "yq": yt[b][:, csl],
            "wp32": pay32[c * PC32:(c + 1) * PC32],
            "wp16": pay16[c * PC16:(c + 1) * PC16],
        })
    return in_maps


def kernel(**inputs):
    if "nc" not in _CACHED:
        _CACHED["nc"] = _build()
    nc = _CACHED["nc"]
    in_maps = _prep_inputs(inputs)
    import time as _time
    _t0 = _time.time()
    res = run_bass_kernel_spmd(nc, in_maps, list(range(8)))
    _CACHED["exec_wall"] = _time.time() - _t0
    _CACHED["last_res"] = res
    out = np.empty((B, S, D2), np.float32)
    for c in range(8):
        b, h = c // 2, c % 2
        out[b, h * SH:(h + 1) * SH, :] = res.results[c]["out"]
    return out


# revision 8
# speedup vs baseline: 4.2076x; 1.7392x over previous
"""Trainium2 Bass kernel for nn_Cross_attention_dl_91061896610498.

Wall-clock through the axon tunnel is dominated by host->device bytes, so
each core uploads only unique data: its query-half of x/y (fp32,
pre-transposed) plus 1/8 of a packed weight payload.  On-device AllGathers
rebuild the full tensors (pair groups for x/y and the stage-1 -> stage-2
temp, 8-way for weights).  fp16 hi/lo splits are computed on device; the
three matmul pair-products per fp32 matmul keep Q/K/score accuracy (no
1/sqrt(d) scaling -> near-one-hot softmax).  Stage 1 is pair-split (each
core computes its query-half of x1/y1), stage 2 runs on the core's half.
Output returns as fp16.

Core c = (batch b=c//2, half h=c%2).  Core uploads:
  xq/yq  [512,1024] f32  - x[b].T columns [h*1024:(h+1)*1024]
  wp32   [353,1024] f32  - 1/8 slice of fp32 payload (W1qkv^T, W2qk^T, biases)
  wp16   [128,1024] f16  - 1/8 slice of W2v^T fp16
"""

import os
import tempfile

import numpy as np

import jax

# cache the XLA executable across calls: run_bass_kernel_spmd re-traces a
# fresh closure per call, which otherwise recompiles the wrapper each time
_cache_dir = os.path.join(tempfile.gettempdir(), "jax_cc_cache")
jax.config.update("jax_compilation_cache_dir", _cache_dir)
jax.config.update("jax_persistent_cache_min_entry_size_bytes", 0)
jax.config.update("jax_persistent_cache_min_compile_time_secs", 0.0)

import concourse.bass as bass
import concourse.mybir as mybir
from concourse.tile import TileContext
from concourse.bass_utils import run_bass_kernel_spmd

F16 = mybir.dt.float16
F32 = mybir.dt.float32
AF = mybir.ActivationFunctionType
ALU = mybir.AluOpType
AX = mybir.AxisListType

D1, D2, B, S = 512, 1024, 4, 2048
SH = S // 2          # per-core query half
QT = 128             # query tile
NQ = SH // QT        # q tiles per core (8, both stages)
NC1 = D1 // 128      # 4 partition chunks of D1
NC2 = D2 // 128      # 8 partition chunks of D2
NKC = S // 128       # 16 key chunks
NSC = S // 512       # 4 moving chunks over S

# fp32 payload layout (rows of 1024 f32)
OW1Q, OW1K, OW1V = 0, 256, 512          # [512,512] each, flat as [256,1024]
OW2Q, OW2K = 768, 1792                  # [1024,1024] each
OB1Q, OB1K, OB1V = 2816, 2817, 2818    # bias rows
OB2Q, OB2K, OB2V, OWRES = 2819, 2820, 2821, 2822
PR32 = 2824                             # padded to /8
PC32 = PR32 // 8                        # 353 rows per core
PR16 = 1024                             # w2v^T rows
PC16 = PR16 // 8                        # 128 rows per core

PAIRS = [[0, 1], [2, 3], [4, 5], [6, 7]]
FULL = [list(range(8))]

_CACHED = {}


def _fix_excess_waits(nc, max_waits=1):
    """walrus in this env accepts only 1 sync-wait per instruction; move
    excess waits onto preceding same-engine NOPs."""
    ctr = 0
    for fn in nc.m.functions:
        for blk in fn.blocks:
            insts = blk.bb.instructions if hasattr(blk, "bb") else blk.instructions
            new = []
            changed = False
            for inst in insts:
                si = inst.sync_info
                waits = list(si.on_wait) if (si is not None and si.on_wait) else []
                if len(waits) > max_waits:
                    excess, keep = waits[:-max_waits], waits[-max_waits:]
                    while excess:
                        chunk, excess = excess[:max_waits], excess[max_waits:]
                        ctr += 1
                        nop = mybir.InstNoOp(name=f"I-waitfix-{ctr}", engine=inst.engine)
                        nop.sync_info = mybir.SyncInfo(on_wait=chunk, on_update=[])
                        new.append(nop)
                    inst.sync_info = mybir.SyncInfo(
                        on_wait=keep,
                        on_update=list(si.on_update) if si.on_update else [],
                    )
                    changed = True
                new.append(inst)
            if changed:
                if hasattr(blk, "bb"):
                    blk.bb.instructions = new
                else:
                    blk.instructions = new
    return ctr


def _pair_mms(nc, psum, lhs_pair, rhs_pair, start, stop=False):
    """Accumulate (lhs_hi+lhs_lo).T @ (rhs_hi+rhs_lo) into psum (lo*lo dropped)."""
    lh, ll = lhs_pair
    rh, rl = rhs_pair
    nc.tensor.matmul(psum, lh, rh, start=start, stop=False)
    nc.tensor.matmul(psum, lh, rl, start=False, stop=False)
    nc.tensor.matmul(psum, ll, rh, start=False, stop=stop)


def _split_rows(nc, pool, wkp, src_ap_fn, nrows, ncols, tag):
    """Load fp32 DRAM rows -> f16 hi/lo SBUF tile pairs [nrows//128][128, ncols].

    src_ap_fn(i) gives the DRAM AP for rows [i*128:(i+1)*128].
    """
    his, los = [], []
    for i in range(nrows // 128):
        st = wkp.tile([128, ncols], F32, tag=f"st{ncols}")
        nc.sync.dma_start(out=st[:], in_=src_ap_fn(i))
        th = pool.tile([128, ncols], F16, tag=f"{tag}_h{i}")
        tl = pool.tile([128, ncols], F16, tag=f"{tag}_l{i}")
        nc.vector.tensor_copy(th[:], st[:])
        nc.vector.tensor_tensor(tl[:], st[:], th[:], op=ALU.subtract)
        his.append(th)
        los.append(tl)
    return his, los


def _softmax_ptiles(nc, pp1, pp2, wkp, sps_h, tag, pair):
    """negmax -> exp (+row sums) -> fp16 (pair) split -> transposed halves."""
    nm = [wkp.tile([128, 1], F32, name=f"nm{tag}{h}", tag=f"nm{tag}{h}") for h in range(2)]
    ls = [wkp.tile([128, 1], F32, name=f"ls{tag}{h}", tag=f"ls{tag}{h}") for h in range(2)]
    pth_halves, ptl_halves = [], []
    for h in range(2):
        nc.vector.reduce_max(nm[h][:], sps_h[h][:], axis=AX.X, negate=True)
        pf = pp1.tile([128, S // 2], F32, tag=f"pf{tag}")
        nc.scalar.activation(pf[:], sps_h[h][:], AF.Exp,
                             bias=nm[h][:, 0:1], accum_out=ls[h][:])
        p_hi = pp1.tile([128, S // 2], F16, tag=f"phi{tag}")
        nc.scalar.copy(p_hi[:], pf[:])
        pth = pp2.tile([128, NKC // 2, 128], F16, tag=f"pth{tag}")
        nc.sync.dma_start_transpose(pth[:], p_hi[:])
        pth_halves.append(pth)
        if pair:
            p_lo = pp1.tile([128, S // 2], F16, tag=f"plo{tag}")
            nc.vector.tensor_tensor(p_lo[:], pf[:], p_hi[:], op=ALU.subtract)
            ptl = pp2.tile([128, NKC // 2, 128], F16, tag=f"ptl{tag}")
            nc.sync.dma_start_transpose(ptl[:], p_lo[:])
            ptl_halves.append(ptl)
    negm = wkp.tile([128, 1], F32, tag=f"negm{tag}")
    nc.vector.tensor_tensor(negm[:], nm[0][:], nm[1][:], op=ALU.min)
    sh = []
    lw = [wkp.tile([128, 1], F32, name=f"lw{tag}{h}", tag=f"lw{tag}{h}") for h in range(2)]
    for h in range(2):
        d = wkp.tile([128, 1], F32, name=f"d{tag}{h}", tag=f"d{tag}{h}")
        nc.vector.tensor_tensor(d[:], negm[:], nm[h][:], op=ALU.subtract)  # m_h - m <= 0
        s = wkp.tile([128, 1], F32, name=f"sh{tag}{h}", tag=f"sh{tag}{h}")
        nc.scalar.activation(s[:], d[:], AF.Exp)
        sh.append(s)
        nc.vector.tensor_tensor(lw[h][:], ls[h][:], s[:], op=ALU.mult)
    lsum = wkp.tile([128, 1], F32, tag=f"lsum{tag}")
    nc.vector.tensor_tensor(lsum[:], lw[0][:], lw[1][:], op=ALU.add)
    rl = wkp.tile([128, 1], F32, tag=f"rl{tag}")
    nc.vector.reciprocal(rl[:], lsum[:])
    c = []
    for h in range(2):
        ch = wkp.tile([128, 1], F32, name=f"c{tag}{h}", tag=f"c{tag}{h}")
        nc.vector.tensor_tensor(ch[:], sh[h][:], rl[:], op=ALU.mult)
        c.append(ch)
    return pth_halves, ptl_halves, c


def _build():
    import concourse.tile_utils as tile_utils
    tile_utils.max_sbuf_usage = 204 * 1024

    nc = bass.Bass("TRN2", target_bir_lowering=False, debug=False)

    xq = nc.dram_tensor("xq", [D1, SH], F32, kind="ExternalInput")
    yq = nc.dram_tensor("yq", [D1, SH], F32, kind="ExternalInput")
    wp32 = nc.dram_tensor("wp32", [PC32, 1024], F32, kind="ExternalInput")
    wp16 = nc.dram_tensor("wp16", [PC16, 1024], F16, kind="ExternalInput")
    out = nc.dram_tensor("out", [SH, D2], F16, kind="ExternalOutput")

    # collective bounce buffers (collectives can't touch External tensors)
    xb = nc.dram_tensor("xb", [D1, SH], F32)
    yb = nc.dram_tensor("yb", [D1, SH], F32)
    w32b = nc.dram_tensor("w32b", [PC32, 1024], F32)
    w16b = nc.dram_tensor("w16b", [PC16, 1024], F16)
    # gathered: xg/yg rows [h*512:(h+1)*512] = x^T cols [h*1024:(h+1)*1024]
    xg = nc.dram_tensor("xg", [2 * D1, SH], F32)
    yg = nc.dram_tensor("yg", [2 * D1, SH], F32)
    w32g = nc.dram_tensor("w32g", [PR32, 1024], F32, addr_space="Shared")
    w16g = nc.dram_tensor("w16g", [PR16, 1024], F16, addr_space="Shared")

    # my tempT half (stage-1 outputs, transposed: [D1, my 1024 q cols])
    x1h_d = nc.dram_tensor("x1h_d", [D1, SH], F16)
    x1l_d = nc.dram_tensor("x1l_d", [D1, SH], F16)
    y1h_d = nc.dram_tensor("y1h_d", [D1, SH], F16)
    y1l_d = nc.dram_tensor("y1l_d", [D1, SH], F16)
    # pair-gathered tempT: rows [h*512:(h+1)*512] = cols [h*1024:(h+1)*1024]
    tgxh = nc.dram_tensor("tgxh", [2 * D1, SH], F16)
    tgxl = nc.dram_tensor("tgxl", [2 * D1, SH], F16)
    tgyh = nc.dram_tensor("tgyh", [2 * D1, SH], F16)
    tgyl = nc.dram_tensor("tgyl", [2 * D1, SH], F16)

    def cc(groups, i, o):
        nc.gpsimd.collective_compute(
            "AllGather", ALU.bypass, replica_groups=groups,
            ins=[i[:].opt()], outs=[o[:].opt()],
        )

    with TileContext(nc) as tc:
        for b_, s_ in [(xb, xq), (yb, yq), (w32b, wp32), (w16b, wp16)]:
            nc.gpsimd.dma_start(out=b_[:], in_=s_[:])
        cc(PAIRS, xb, xg)
        cc(PAIRS, yb, yg)
        cc(FULL, w32b, w32g)
        cc(FULL, w16b, w16g)

        # payload views
        v512 = w32g.rearrange("r (k c) -> (r k) c", k=2)       # [5648, 512]
        vp8 = w32g.rearrange("r (p j) -> p r j", p=128)        # [128, 2824, 8]

        with tc.tile_pool(name="const", bufs=1) as cp:
            b1q_sb = cp.tile([128, 1, 8], F32, tag="b1q")
            b1k_sb = cp.tile([128, 1, 8], F32, tag="b1k")
            b2q_sb = cp.tile([128, 1, 8], F32, tag="b2q")
            b2k_sb = cp.tile([128, 1, 8], F32, tag="b2k")
            wres_sb = cp.tile([128, 1, 8], F32, tag="wres")
            for sb, row in [(b1q_sb, OB1Q), (b1k_sb, OB1K), (b2q_sb, OB2Q),
                            (b2k_sb, OB2K), (wres_sb, OWRES)]:
                nc.sync.dma_start(out=sb[:], in_=vp8[:, row:row + 1, :])
            b1v_sb = (cp.tile([1, D1], F16, name="b1vh", tag="b1vh"),
                      cp.tile([1, D1], F16, name="b1vl", tag="b1vl"))
            b2v_sb = (cp.tile([1, D2], F16, name="b2vh", tag="b2vh"),
                      cp.tile([1, D2], F16, name="b2vl", tag="b2vl"))
            with tc.tile_pool(name="cwk", bufs=1) as cwk:
                b1v_f = cwk.tile([1, D1], F32, tag="b1vf")
                b2v_f = cwk.tile([1, D2], F32, tag="b2vf")
                nc.sync.dma_start(out=b1v_f[:], in_=w32g[OB1V:OB1V + 1, 0:D1])
                nc.sync.dma_start(out=b2v_f[:], in_=w32g[OB2V:OB2V + 1, :])
                for (th, tl), tf in [(b1v_sb, b1v_f), (b2v_sb, b2v_f)]:
                    nc.vector.tensor_copy(th[:], tf[:])
                    nc.vector.tensor_tensor(tl[:], tf[:], th[:], op=ALU.subtract)
            ones_sb = cp.tile([1, 128], F16, tag="ones1")
            nc.vector.memset(ones_sb[:], 1.0)

            # ---------------- stage 1 ----------------
            with tc.tile_pool(name="loc", bufs=1) as locp:
                with tc.tile_pool(name="lwk", bufs=2) as lwk:
                    # W1^T chunk pairs [t][dc][128, 512]
                    w1sb = {}
                    for t, off in [("q", OW1Q), ("k", OW1K), ("v", OW1V)]:
                        w1sb[t] = _split_rows(
                            nc, locp, lwk,
                            lambda i, o=off: v512[2 * o + i * 128:2 * o + (i + 1) * 128, :],
                            D1, D1, f"w1{t}")
                    # local q-half pairs [dc][128, 1024]
                    xql = _split_rows(nc, locp, lwk,
                                      lambda i: xq[i * 128:(i + 1) * 128, :],
                                      D1, SH, "xql")
                    yql = _split_rows(nc, locp, lwk,
                                      lambda i: yq[i * 128:(i + 1) * 128, :],
                                      D1, SH, "yql")

                for ti, (src_g, q_loc, r_loc, wcol, o_hi, o_lo) in enumerate([
                        (xg, xql, yql, 0, x1h_d, x1l_d),
                        (yg, yql, xql, 1, y1h_d, y1l_d)]):
                    _stage1_attn(nc, tc, ti, src_g, q_loc, r_loc, wcol, o_hi, o_lo,
                                 w1sb, b1q_sb, b1k_sb, b1v_sb, ones_sb, wres_sb)

            cc(PAIRS, x1h_d, tgxh)
            cc(PAIRS, y1h_d, tgyh)
            cc(PAIRS, x1l_d, tgxl)
            cc(PAIRS, y1l_d, tgyl)

            # ---------------- stage 2 ----------------
            _stage2(nc, tc, (tgxh, tgxl, tgyh, tgyl),
                    (x1h_d, x1l_d, y1h_d, y1l_d), w32g, w16g,
                    b2q_sb, b2k_sb, b2v_sb, ones_sb, out)

    _fix_excess_waits(nc)
    return nc


def _stage1_attn(nc, tc, ti, src_g, q_loc, r_loc, wcol, o_hi, o_lo,
                 w1sb, b1q_sb, b1k_sb, b1v_sb, ones_sb, wres_sb):
    q_hi_loc, q_lo_loc = q_loc
    r_hi_loc, r_lo_loc = r_loc
    with (tc.tile_pool(name=f"kv{ti}", bufs=1) as kvp,
          tc.tile_pool(name=f"wk{ti}", bufs=2) as wkp,
          tc.tile_pool(name=f"ps{ti}", bufs=4, space="PSUM") as pp,
          tc.tile_pool(name=f"sc{ti}", bufs=2, space="PSUM") as scp):
        kt_hi = [kvp.tile([128, S], F16, name=f"kth{ec}", tag=f"kth{ec}")
                 for ec in range(NC1)]
        kt_lo = [kvp.tile([128, S], F16, name=f"ktl{ec}", tag=f"ktl{ec}")
                 for ec in range(NC1)]
        v_hi = [kvp.tile([128, D1], F16, name=f"vh{kc}", tag=f"vh{kc}")
                for kc in range(NKC)]
        v_lo = [kvp.tile([128, D1], F16, name=f"vl{kc}", tag=f"vl{kc}")
                for kc in range(NKC)]

        # full-sequence source pairs (scoped: freed before the q loop)
        with tc.tile_pool(name=f"src{ti}", bufs=1) as srcp, \
             tc.tile_pool(name=f"swk{ti}", bufs=2) as swk:
            src_hi, src_lo = [], []
            for dc in range(NC1):
                th = srcp.tile([128, S], F16, name=f"sfh{dc}", tag=f"sfh{dc}")
                tl = srcp.tile([128, S], F16, name=f"sfl{dc}", tag=f"sfl{dc}")
                for hh in range(2):
                    st = swk.tile([128, SH], F32, tag="st_src")
                    nc.sync.dma_start(
                        out=st[:],
                        in_=src_g[hh * D1 + dc * 128:hh * D1 + (dc + 1) * 128, :])
                    csl = slice(hh * SH, (hh + 1) * SH)
                    nc.vector.tensor_copy(th[:, csl], st[:])
                    nc.vector.tensor_tensor(tl[:, csl], st[:], th[:, csl],
                                            op=ALU.subtract)
                src_hi.append(th)
                src_lo.append(tl)

            # K^T pair [ec][128, S]
            for ec in range(NC1):
                for sc in range(NSC):
                    ssl = slice(sc * 512, (sc + 1) * 512)
                    ps = pp.tile([128, 512], F32, tag="ps")
                    for dc in range(NC1):
                        _pair_mms(nc, ps[:],
                                  (w1sb["k"][0][dc][:, ec * 128:(ec + 1) * 128],
                                   w1sb["k"][1][dc][:, ec * 128:(ec + 1) * 128]),
                                  (src_hi[dc][:, ssl], src_lo[dc][:, ssl]),
                                  start=(dc == 0))
                    kf = wkp.tile([128, 512], F32, tag="kevac")
                    nc.vector.tensor_scalar(kf[:], ps[:], b1k_sb[:, 0, ec:ec + 1],
                                            None, op0=ALU.add)
                    nc.vector.tensor_copy(kt_hi[ec][:, ssl], kf[:])
                    nc.vector.tensor_tensor(kt_lo[ec][:, ssl], kf[:], kt_hi[ec][:, ssl],
                                            op=ALU.subtract)

            # V pair [kc][128, D1]; bias via rank-1 ones x b1v
            for kc in range(NKC):
                ps = pp.tile([128, 512], F32, tag="ps")
                nc.tensor.matmul(ps[:], ones_sb[:], b1v_sb[0][:], start=True, stop=False)
                nc.tensor.matmul(ps[:], ones_sb[:], b1v_sb[1][:], start=False, stop=False)
                for dc in range(NC1):
                    _pair_mms(nc, ps[:],
                              (src_hi[dc][:, kc * 128:(kc + 1) * 128],
                               src_lo[dc][:, kc * 128:(kc + 1) * 128]),
                              (w1sb["v"][0][dc][:], w1sb["v"][1][dc][:]),
                              start=False)
                nc.vector.tensor_copy(v_hi[kc][:], ps[:])
                nc.vector.tensor_tensor(v_lo[kc][:], ps[:], v_hi[kc][:], op=ALU.subtract)

        with (tc.tile_pool(name=f"pa{ti}", bufs=1) as ptp1,
              tc.tile_pool(name=f"pt{ti}", bufs=2) as ptp2):
            _stage1_qloop(nc, qi_pool=(ptp1, ptp2, pp, scp, wkp), w1sb=w1sb,
                          q_loc=(q_hi_loc, q_lo_loc), r_loc=(r_hi_loc, r_lo_loc),
                          kt=(kt_hi, kt_lo), v=(v_hi, v_lo), wcol=wcol,
                          b1q_sb=b1q_sb, wres_sb=wres_sb, o_hi=o_hi, o_lo=o_lo)


def _stage1_qloop(nc, qi_pool, w1sb, q_loc, r_loc, kt, v, wcol,
                  b1q_sb, wres_sb, o_hi, o_lo):
    ptp1, ptp2, pp, scp, wkp = qi_pool
    q_hi_loc, q_lo_loc = q_loc
    r_hi_loc, r_lo_loc = r_loc
    kt_hi, kt_lo = kt
    v_hi, v_lo = v
    if True:
        for qi in range(NQ):
            qsl = slice(qi * QT, (qi + 1) * QT)
            # Q^T for this tile from LOCAL half: psum [128, 4*128]
            qps = pp.tile([128, 512], F32, tag="ps")
            for ec in range(NC1):
                for dc in range(NC1):
                    _pair_mms(nc, qps[:, ec * 128:(ec + 1) * 128],
                              (w1sb["q"][0][dc][:, ec * 128:(ec + 1) * 128],
                               w1sb["q"][1][dc][:, ec * 128:(ec + 1) * 128]),
                              (q_hi_loc[dc][:, qsl], q_lo_loc[dc][:, qsl]),
                              start=(dc == 0))
            qf = wkp.tile([128, 512], F32, tag="qevac")
            for ec in range(NC1):
                esl = slice(ec * 128, (ec + 1) * 128)
                nc.vector.tensor_scalar(qf[:, esl], qps[:, esl],
                                        b1q_sb[:, 0, ec:ec + 1], None, op0=ALU.add)
            q_hi = wkp.tile([128, 512], F16, tag="qhi")
            q_lo = wkp.tile([128, 512], F16, tag="qlo")
            nc.vector.tensor_copy(q_hi[:], qf[:])
            nc.vector.tensor_tensor(q_lo[:], qf[:], q_hi[:], op=ALU.subtract)

            sps_h = [scp.tile([128, S // 2], F32, name=f"scr{h}", tag="scoresh")
                     for h in range(2)]
            for sc in range(NSC):
                ssl = slice(sc * 512, (sc + 1) * 512)
                hsl = slice((sc % 2) * 512, (sc % 2) * 512 + 512)
                for ec in range(NC1):
                    esl = slice(ec * 128, (ec + 1) * 128)
                    _pair_mms(nc, sps_h[sc // 2][:, hsl],
                              (q_hi[:, esl], q_lo[:, esl]),
                              (kt_hi[ec][:, ssl], kt_lo[ec][:, ssl]),
                              start=(ec == 0))

            pth, ptl, c = _softmax_ptiles(nc, ptp1, ptp2, wkp, sps_h, "1", pair=True)

            ops_h = []
            for h in range(2):
                ops = pp.tile([128, 512], F32, name=f"av{h}", tag="ps")
                for kc8 in range(NKC // 2):
                    kc = h * (NKC // 2) + kc8
                    nc.tensor.matmul(ops[:], pth[h][:, kc8, :], v_hi[kc][:],
                                     start=(kc8 == 0), stop=False)
                    nc.tensor.matmul(ops[:], pth[h][:, kc8, :], v_lo[kc][:],
                                     start=False, stop=False)
                    nc.tensor.matmul(ops[:], ptl[h][:, kc8, :], v_hi[kc][:],
                                     start=False, stop=(kc8 == NKC // 2 - 1))
                ops_h.append(ops)

            af = ptp1.tile([128, 512], F32, tag="af")
            nc.vector.tensor_scalar(af[:], ops_h[0][:], c[0][:, 0:1], None, op0=ALU.mult)
            af2 = ptp1.tile([128, 512], F32, tag="af2")
            nc.vector.tensor_scalar(af2[:], ops_h[1][:], c[1][:, 0:1], None, op0=ALU.mult)
            nc.vector.tensor_tensor(af[:], af[:], af2[:], op=ALU.add)
            a_hi = wkp.tile([128, 512], F16, tag="ahi")
            a_lo = wkp.tile([128, 512], F16, tag="alo")
            nc.scalar.copy(a_hi[:], af[:])
            nc.vector.tensor_tensor(a_lo[:], af[:], a_hi[:], op=ALU.subtract)
            at_hi = wkp.tile([128, NC1, 128], F16, tag="athi")
            at_lo = wkp.tile([128, NC1, 128], F16, tag="atlo")
            nc.sync.dma_start_transpose(at_hi[:], a_hi[:])
            nc.sync.dma_start_transpose(at_lo[:], a_lo[:])

            # residual in transposed space, then resplit; single strided store
            x1h = wkp.tile([128, NC1, 128], F16, tag="x1h")
            x1l = wkp.tile([128, NC1, 128], F16, tag="x1l")
            for ec in range(NC1):
                r1 = wkp.tile([128, 128], F32, tag="r1")
                nc.vector.tensor_scalar(r1[:], r_hi_loc[ec][:, qsl],
                                        wres_sb[:, 0, wcol:wcol + 1], None, op0=ALU.mult)
                nc.vector.tensor_tensor(r1[:], r1[:], at_hi[:, ec, :], op=ALU.add)
                r2 = wkp.tile([128, 128], F32, tag="r2")
                nc.vector.tensor_scalar(r2[:], r_lo_loc[ec][:, qsl],
                                        wres_sb[:, 0, wcol:wcol + 1], None, op0=ALU.mult)
                nc.vector.tensor_tensor(r2[:], r2[:], at_lo[:, ec, :], op=ALU.add)
                nc.vector.tensor_tensor(r1[:], r1[:], r2[:], op=ALU.add)
                nc.scalar.copy(x1h[:, ec, :], r1[:])
                nc.vector.tensor_tensor(x1l[:, ec, :], r1[:], x1h[:, ec, :], op=ALU.subtract)
            oh_ap = o_hi.rearrange("(c p) q -> p c q", p=128)[:, :, qsl]
            ol_ap = o_lo.rearrange("(c p) q -> p c q", p=128)[:, :, qsl]
            nc.gpsimd.dma_start(out=oh_ap, in_=x1h[:])
            nc.gpsimd.dma_start(out=ol_ap, in_=x1l[:])


def _stage2(nc, tc, tg, tloc, w32g, w16g, b2q_sb, b2k_sb, b2v_sb, ones_sb, out):
    tgxh, tgxl, tgyh, tgyl = tg
    x1h_d, x1l_d, y1h_d, y1l_d = tloc

    def tt_gath(dc, hi, shalf):
        if dc < NC1:
            dr = tgxh if hi else tgxl
        else:
            dr = tgyh if hi else tgyl
        r = shalf * D1 + (dc % NC1) * 128
        return dr[r:r + 128, :]

    def tt_loc(dc, hi):
        if dc < NC1:
            dr = x1h_d if hi else x1l_d
        else:
            dr = y1h_d if hi else y1l_d
        r = (dc % NC1) * 128
        return dr[r:r + 128, :]

    with (tc.tile_pool(name="s2", bufs=1) as s2p,
          tc.tile_pool(name="s2wk", bufs=2) as wkp,
          tc.tile_pool(name="s2pa", bufs=1) as ptp1,
          tc.tile_pool(name="s2pt", bufs=2) as ptp2,
          tc.tile_pool(name="s2ps", bufs=2, space="PSUM") as pp,
          tc.tile_pool(name="s2sc", bufs=2, space="PSUM") as scp):
        # V2 single fp16 [kc][128, D2]; temp-lo dropped; bias via rank-1
        v2 = []
        with tc.tile_pool(name="w2vp", bufs=1) as wp, \
             tc.tile_pool(name="ttv", bufs=2) as ttp:
            wv = []
            for i in range(NC2):
                t = wp.tile([128, D2], F16, tag=f"w2v{i}")
                nc.gpsimd.dma_start(out=t[:], in_=w16g[i * 128:(i + 1) * 128, :])
                wv.append(t)
            for kcg in range(NKC // 4):
                lsl_g = slice((kcg % 2) * 512, (kcg % 2) * 512 + 512)
                tchunks = []
                for dc in range(NC2):
                    t = ttp.tile([128, 512], F16, tag=f"ttv{dc}")
                    nc.gpsimd.dma_start(out=t[:], in_=tt_gath(dc, True, kcg // 2)[:, lsl_g])
                    tchunks.append(t)
                for kcl in range(4):
                    kc = kcg * 4 + kcl
                    lsl = slice(kcl * 128, (kcl + 1) * 128)
                    vt = s2p.tile([128, D2], F16, name=f"v2_{kc}", tag=f"v2{kc}")
                    for e2c in range(2):
                        esl = slice(e2c * 512, (e2c + 1) * 512)
                        ps = pp.tile([128, 512], F32, tag="ps2")
                        nc.tensor.matmul(ps[:], ones_sb[:], b2v_sb[0][:, esl],
                                         start=True, stop=False)
                        nc.tensor.matmul(ps[:], ones_sb[:], b2v_sb[1][:, esl],
                                         start=False, stop=False)
                        for dc in range(NC2):
                            nc.tensor.matmul(ps[:], tchunks[dc][:, lsl], wv[dc][:, esl],
                                             start=False, stop=(dc == NC2 - 1))
                        nc.vector.tensor_copy(vt[:, esl], ps[:])
                    v2.append(vt)

        # K2^T pair [ec][128, S]; gathered tempT pair + W2k^T f32 streamed
        k2_hi = [s2p.tile([128, S], F16, name=f"k2h{ec}", tag=f"k2h{ec}") for ec in range(NC2)]
        k2_lo = [s2p.tile([128, S], F16, name=f"k2l{ec}", tag=f"k2l{ec}") for ec in range(NC2)]
        with tc.tile_pool(name="w2ks", bufs=1) as wks, \
             tc.tile_pool(name="ttk", bufs=1) as ttp, \
             tc.tile_pool(name="wst", bufs=2) as wst:
            for sc in range(NSC):
                ssl = slice(sc * 512, (sc + 1) * 512)
                lsl_g = slice((sc % 2) * 512, (sc % 2) * 512 + 512)
                tch, tcl = [], []
                for dc in range(NC2):
                    th = ttp.tile([128, 512], F16, tag=f"ttkh{dc}")
                    tl = ttp.tile([128, 512], F16, tag=f"ttkl{dc}")
                    nc.gpsimd.dma_start(out=th[:], in_=tt_gath(dc, True, sc // 2)[:, lsl_g])
                    nc.gpsimd.dma_start(out=tl[:], in_=tt_gath(dc, False, sc // 2)[:, lsl_g])
                    tch.append(th)
                    tcl.append(tl)
                for e2h in range(2):
                    wsl = slice(e2h * 512, (e2h + 1) * 512)
                    wrh, wrl = [], []
                    for dc in range(NC2):
                        st = wst.tile([128, 512], F32, tag="wkst")
                        nc.sync.dma_start(
                            out=st[:],
                            in_=w32g[OW2K + dc * 128:OW2K + (dc + 1) * 128, wsl])
                        wh = wks.tile([128, 512], F16, name=f"wkh{dc}", tag=f"wkh{dc}")
                        wl = wks.tile([128, 512], F16, name=f"wkl{dc}", tag=f"wkl{dc}")
                        nc.vector.tensor_copy(wh[:], st[:])
                        nc.vector.tensor_tensor(wl[:], st[:], wh[:], op=ALU.subtract)
                        wrh.append(wh)
                        wrl.append(wl)
                    for ecl in range(4):
                        ec = e2h * 4 + ecl
                        lsl = slice(ecl * 128, (ecl + 1) * 128)
                        ps = pp.tile([128, 512], F32, tag="ps2")
                        for dc in range(NC2):
                            _pair_mms(nc, ps[:],
                                      (wrh[dc][:, lsl], wrl[dc][:, lsl]),
                                      (tch[dc][:], tcl[dc][:]),
                                      start=(dc == 0))
                        kf = wkp.tile([128, 512], F32, tag="k2evac")
                        nc.vector.tensor_scalar(kf[:], ps[:], b2k_sb[:, 0, ec:ec + 1],
                                                None, op0=ALU.add)
                        nc.vector.tensor_copy(k2_hi[ec][:, ssl], kf[:])
                        nc.vector.tensor_tensor(k2_lo[ec][:, ssl], kf[:], k2_hi[ec][:, ssl],
                                                op=ALU.subtract)

        # Q2^T pair for my SH query columns, from LOCAL tempT
        q2_hi = [s2p.tile([128, SH], F16, name=f"q2h{ec}", tag=f"q2h{ec}") for ec in range(NC2)]
        q2_lo = [s2p.tile([128, SH], F16, name=f"q2l{ec}", tag=f"q2l{ec}") for ec in range(NC2)]
        with tc.tile_pool(name="w2qs", bufs=1) as wqs, \
             tc.tile_pool(name="ttq", bufs=1) as ttp, \
             tc.tile_pool(name="wsq", bufs=2) as wst:
            for sc in range(SH // 512):
                ssl = slice(sc * 512, (sc + 1) * 512)
                tch, tcl = [], []
                for dc in range(NC2):
                    th = ttp.tile([128, 512], F16, tag=f"ttqh{dc}")
                    tl = ttp.tile([128, 512], F16, tag=f"ttql{dc}")
                    nc.gpsimd.dma_start(out=th[:], in_=tt_loc(dc, True)[:, ssl])
                    nc.gpsimd.dma_start(out=tl[:], in_=tt_loc(dc, False)[:, ssl])
                    tch.append(th)
                    tcl.append(tl)
                for e2h in range(2):
                    wsl = slice(e2h * 512, (e2h + 1) * 512)
                    wrh, wrl = [], []
                    for dc in range(NC2):
                        st = wst.tile([128, 512], F32, tag="wqst")
                        nc.sync.dma_start(
                            out=st[:],
                            in_=w32g[OW2Q + dc * 128:OW2Q + (dc + 1) * 128, wsl])
                        wh = wqs.tile([128, 512], F16, name=f"wqh{dc}", tag=f"wqh{dc}")
                        wl = wqs.tile([128, 512], F16, name=f"wql{dc}", tag=f"wql{dc}")
                        nc.vector.tensor_copy(wh[:], st[:])
                        nc.vector.tensor_tensor(wl[:], st[:], wh[:], op=ALU.subtract)
                        wrh.append(wh)
                        wrl.append(wl)
                    for ecl in range(4):
                        ec = e2h * 4 + ecl
                        lsl = slice(ecl * 128, (ecl + 1) * 128)
                        ps = pp.tile([128, 512], F32, tag="ps2")
                        for dc in range(NC2):
                            _pair_mms(nc, ps[:],
                                      (wrh[dc][:, lsl], wrl[dc][:, lsl]),
                                      (tch[dc][:], tcl[dc][:]),
                                      start=(dc == 0))
                        qf = wkp.tile([128, 512], F32, tag="q2evac")
                        nc.vector.tensor_scalar(qf[:], ps[:], b2q_sb[:, 0, ec:ec + 1],
                                                None, op0=ALU.add)
                        nc.vector.tensor_copy(q2_hi[ec][:, ssl], qf[:])
                        nc.vector.tensor_tensor(q2_lo[ec][:, ssl], qf[:], q2_hi[ec][:, ssl],
                                                op=ALU.subtract)

        # attention over my 8 q-tiles
        for qi in range(NQ):
            qsl = slice(qi * QT, (qi + 1) * QT)
            sps_h = [scp.tile([128, S // 2], F32, name=f"s2scr{h}", tag="s2scoresh")
                     for h in range(2)]
            for sc in range(NSC):
                ssl = slice(sc * 512, (sc + 1) * 512)
                hsl = slice((sc % 2) * 512, (sc % 2) * 512 + 512)
                for ec in range(NC2):
                    _pair_mms(nc, sps_h[sc // 2][:, hsl],
                              (q2_hi[ec][:, qsl], q2_lo[ec][:, qsl]),
                              (k2_hi[ec][:, ssl], k2_lo[ec][:, ssl]),
                              start=(ec == 0))

            pth, _, c = _softmax_ptiles(nc, ptp1, ptp2, wkp, sps_h, "2", pair=False)

            ops_h = []
            for h in range(2):
                ops = pp.tile([128, D2], F32, name=f"av2{h}", tag="ps2")
                for e2c in range(2):
                    esl = slice(e2c * 512, (e2c + 1) * 512)
                    for kc8 in range(NKC // 2):
                        kc = h * (NKC // 2) + kc8
                        nc.tensor.matmul(ops[:, esl], pth[h][:, kc8, :], v2[kc][:, esl],
                                         start=(kc8 == 0), stop=(kc8 == NKC // 2 - 1))
                ops_h.append(ops)
            of = ptp1.tile([128, D2], F32, tag="of2")
            nc.vector.tensor_scalar(of[:], ops_h[0][:], c[0][:, 0:1], None, op0=ALU.mult)
            of2 = ptp1.tile([128, D2], F32, tag="of2b")
            nc.vector.tensor_scalar(of2[:], ops_h[1][:], c[1][:, 0:1], None, op0=ALU.mult)
            nc.vector.tensor_tensor(of[:], of[:], of2[:], op=ALU.add)
            o16 = ptp1.tile([128, D2], F16, tag="o16")
            nc.vector.tensor_copy(o16[:], of[:])
            nc.sync.dma_start(out=out[qsl, :], in_=o16[:])


def _row_p8(vec):
    """Pack vec (len n*128, n<=8) into a 1024-row as flat[p*8+j] = vec[j*128+p]."""
    n = len(vec) // 128
    row = np.zeros((128, 8), np.float32)
    row[:, :n] = np.asarray(vec, np.float32).reshape(n, 128).T
    return row.reshape(1024)


def _prep_payload(inputs):
    pay32 = np.zeros((PR32, 1024), np.float32)
    pay32[OW1Q:OW1Q + 256] = np.ascontiguousarray(
        np.asarray(inputs["sa1_Wq"], np.float32).T).reshape(256, 1024)
    pay32[OW1K:OW1K + 256] = np.ascontiguousarray(
        np.asarray(inputs["sa1_Wk"], np.float32).T).reshape(256, 1024)
    pay32[OW1V:OW1V + 256] = np.ascontiguousarray(
        np.asarray(inputs["sa1_Wv"], np.float32).T).reshape(256, 1024)
    pay32[OW2Q:OW2Q + 1024] = np.asarray(inputs["sa2_Wq"], np.float32).T
    pay32[OW2K:OW2K + 1024] = np.asarray(inputs["sa2_Wk"], np.float32).T
    pay32[OB1Q] = _row_p8(inputs["sa1_bq"])
    pay32[OB1K] = _row_p8(inputs["sa1_bk"])
    pay32[OB1V, 0:D1] = np.asarray(inputs["sa1_bv"], np.float32)
    pay32[OB2Q] = _row_p8(inputs["sa2_bq"])
    pay32[OB2K] = _row_p8(inputs["sa2_bk"])
    pay32[OB2V] = np.asarray(inputs["sa2_bv"], np.float32)
    w1v = float(np.asarray(inputs["weight1"]).reshape(-1)[0])
    w2v = float(np.asarray(inputs["weight2"]).reshape(-1)[0])
    wres = np.zeros((128, 8), np.float32)
    wres[:, 0] = w2v
    wres[:, 1] = w1v
    pay32[OWRES] = wres.reshape(1024)
    pay16 = np.ascontiguousarray(
        np.asarray(inputs["sa2_Wv"], np.float32).T).astype(np.float16)
    return pay32, pay16


def _prep_inputs(inputs):
    x = np.asarray(inputs["x"], np.float32)
    y = np.asarray(inputs["y"], np.float32)
    pay32, pay16 = _prep_payload(inputs)
    xt = [np.ascontiguousarray(x[b].T) for b in range(B)]
    yt = [np.ascontiguousarray(y[b].T) for b in range(B)]
    in_maps = []
    for c in range(8):
        b, h = c // 2, c % 2
        csl = slice(h * SH, (h + 1) * SH)
        in_maps.append({
            "xq": xt[b][:, csl],
            "yq": yt[b][:, csl],
            "wp32": pay32[c * PC32:(c + 1) * PC32],
            "wp16": pay16[c * PC16:(c + 1) * PC16],
        })
    return in_maps


def kernel(**inputs):
    if "nc" not in _CACHED:
        _CACHED["nc"] = _build()
    nc = _CACHED["nc"]
    in_maps = _prep_inputs(inputs)
    import time as _time
    _t0 = _time.time()
    res = run_bass_kernel_spmd(nc, in_maps, list(range(8)))
    _CACHED["exec_wall"] = _time.time() - _t0
    _CACHED["last_res"] = res
    out = np.empty((B, S, D2), np.float32)
    for c in range(8):
        b, h = c // 2, c % 2
        out[b, h * SH:(h + 1) * SH, :] = res.results[c]["out"]
    return out


# revision 22
# speedup vs baseline: 4.6775x; 1.1117x over previous
"""Trainium2 Bass kernel for nn_Cross_attention_dl_91061896610498.

Wall-clock through the axon tunnel is dominated by host->device bytes, so
each core uploads only unique data: its query-half of x/y (fp32,
pre-transposed) plus 1/8 of a packed weight payload.  On-device AllGathers
rebuild the full tensors (pair groups for x/y and the stage-1 -> stage-2
temp, 8-way for weights).  fp16 hi/lo splits are computed on device; the
three matmul pair-products per fp32 matmul keep Q/K/score accuracy (no
1/sqrt(d) scaling -> near-one-hot softmax).  Stage 1 is pair-split (each
core computes its query-half of x1/y1), stage 2 runs on the core's half.
Output returns as fp16.

Core c = (batch b=c//2, half h=c%2).  Core uploads:
  xq/yq  [512,1024] f32  - x[b].T columns [h*1024:(h+1)*1024]
  wp32   [353,1024] f32  - 1/8 slice of fp32 payload (W1qkv^T, W2qk^T, biases)
  wp16   [128,1024] f16  - 1/8 slice of W2v^T fp16
"""

import os
import tempfile

import numpy as np

import jax

# cache the XLA executable across calls: run_bass_kernel_spmd re-traces a
# fresh closure per call, which otherwise recompiles the wrapper each time
_cache_dir = os.path.join(tempfile.gettempdir(), "jax_cc_cache")
jax.config.update("jax_compilation_cache_dir", _cache_dir)
jax.config.update("jax_persistent_cache_min_entry_size_bytes", 0)
jax.config.update("jax_persistent_cache_min_compile_time_secs", 0.0)

import concourse.bass as bass
import concourse.mybir as mybir
from concourse.tile import TileContext
from concourse.bass_utils import run_bass_kernel_spmd

F16 = mybir.dt.float16
F32 = mybir.dt.float32
F8 = mybir.dt.float8e4   # e4m3
LSCALE = 64.0            # lo-residual scale (power of two: f8 -> f16 unscale exact)
AF = mybir.ActivationFunctionType
ALU = mybir.AluOpType
AX = mybir.AxisListType

D1, D2, B, S = 512, 1024, 4, 2048
SH = S // 2          # per-core query half
QT = 128             # query tile
NQ = SH // QT        # q tiles per core (8, both stages)
NC1 = D1 // 128      # 4 partition chunks of D1
NC2 = D2 // 128      # 8 partition chunks of D2
NKC = S // 128       # 16 key chunks
NSC = S // 512       # 4 moving chunks over S

# fp32 payload layout (rows of 1024 f32)
OW1Q, OW1K, OW1V = 0, 256, 512          # [512,512] each, flat as [256,1024]
OW2Q, OW2K = 768, 1792                  # [1024,1024] each
OB1Q, OB1K, OB1V = 2816, 2817, 2818    # bias rows
OB2Q, OB2K, OB2V, OWRES = 2819, 2820, 2821, 2822
PR32 = 2824                             # padded to /8
PC32 = PR32 // 8                        # 353 rows per core
PR16 = 1024                             # w2v^T rows
PC16 = PR16 // 8                        # 128 rows per core

PAIRS = [[0, 1], [2, 3], [4, 5], [6, 7]]
FULL = [list(range(8))]

_CACHED = {}


def _fix_excess_waits(nc, max_waits=1):
    """walrus in this env accepts only 1 sync-wait per instruction; move
    excess waits onto preceding same-engine NOPs."""
    ctr = 0
    for fn in nc.m.functions:
        for blk in fn.blocks:
            insts = blk.bb.instructions if hasattr(blk, "bb") else blk.instructions
            new = []
            changed = False
            for inst in insts:
                si = inst.sync_info
                waits = list(si.on_wait) if (si is not None and si.on_wait) else []
                if len(waits) > max_waits:
                    excess, keep = waits[:-max_waits], waits[-max_waits:]
                    while excess:
                        chunk, excess = excess[:max_waits], excess[max_waits:]
                        ctr += 1
                        nop = mybir.InstNoOp(name=f"I-waitfix-{ctr}", engine=inst.engine)
                        nop.sync_info = mybir.SyncInfo(on_wait=chunk, on_update=[])
                        new.append(nop)
                    inst.sync_info = mybir.SyncInfo(
                        on_wait=keep,
                        on_update=list(si.on_update) if si.on_update else [],
                    )
                    changed = True
                new.append(inst)
            if changed:
                if hasattr(blk, "bb"):
                    blk.bb.instructions = new
                else:
                    blk.instructions = new
    return ctr


def _pair_mms(nc, psum, lhs_pair, rhs_pair, start, stop=False):
    """Accumulate (lhs_hi+lhs_lo).T @ (rhs_hi+rhs_lo) into psum (lo*lo dropped)."""
    lh, ll = lhs_pair
    rh, rl = rhs_pair
    nc.tensor.matmul(psum, lh, rh, start=start, stop=False)
    nc.tensor.matmul(psum, lh, rl, start=False, stop=False)
    nc.tensor.matmul(psum, ll, rh, start=False, stop=stop)


def _split_rows(nc, pool, wkp, src_ap_fn, nrows, ncols, tag):
    """Load fp32 DRAM rows -> f16 hi/lo SBUF tile pairs [nrows//128][128, ncols].

    src_ap_fn(i) gives the DRAM AP for rows [i*128:(i+1)*128].
    """
    his, los = [], []
    for i in range(nrows // 128):
        st = wkp.tile([128, ncols], F32, tag=f"st{ncols}")
        nc.sync.dma_start(out=st[:], in_=src_ap_fn(i))
        th = pool.tile([128, ncols], F16, tag=f"{tag}_h{i}")
        tl = pool.tile([128, ncols], F16, tag=f"{tag}_l{i}")
        nc.vector.tensor_copy(th[:], st[:])
        nc.vector.tensor_tensor(tl[:], st[:], th[:], op=ALU.subtract)
        his.append(th)
        los.append(tl)
    return his, los


def _hilo_rows(nc, pool, wkp, hi_ap_fn, lo_ap_fn, nrows, ncols, tag):
    """Load f16-hi rows directly + f8-lo rows via x(1/LSCALE) convert."""
    his, los = [], []
    for i in range(nrows // 128):
        th = pool.tile([128, ncols], F16, tag=f"{tag}_h{i}")
        tl = pool.tile([128, ncols], F16, tag=f"{tag}_l{i}")
        nc.sync.dma_start(out=th[:], in_=hi_ap_fn(i))
        st8 = wkp.tile([128, ncols], F8, tag=f"st8{ncols}")
        nc.sync.dma_start(out=st8[:], in_=lo_ap_fn(i))
        nc.vector.tensor_scalar(tl[:], st8[:], 1.0 / LSCALE, None, op0=ALU.mult)
        his.append(th)
        los.append(tl)
    return his, los


def _softmax_ptiles(nc, pp1, pp2, wkp, sps_h, tag, pair):
    """negmax -> exp (+row sums) -> fp16 (pair) split -> transposed halves."""
    nm = [wkp.tile([128, 1], F32, name=f"nm{tag}{h}", tag=f"nm{tag}{h}") for h in range(2)]
    ls = [wkp.tile([128, 1], F32, name=f"ls{tag}{h}", tag=f"ls{tag}{h}") for h in range(2)]
    pth_halves, ptl_halves = [], []
    for h in range(2):
        nc.vector.reduce_max(nm[h][:], sps_h[h][:], axis=AX.X, negate=True)
        pf = pp1.tile([128, S // 2], F32, tag=f"pf{tag}")
        nc.scalar.activation(pf[:], sps_h[h][:], AF.Exp,
                             bias=nm[h][:, 0:1], accum_out=ls[h][:])
        p_hi = pp1.tile([128, S // 2], F16, tag=f"phi{tag}")
        nc.scalar.copy(p_hi[:], pf[:])
        pth = pp2.tile([128, NKC // 2, 128], F16, tag=f"pth{tag}")
        nc.sync.dma_start_transpose(pth[:], p_hi[:])
        pth_halves.append(pth)
        if pair:
            p_lo = pp1.tile([128, S // 2], F16, tag=f"plo{tag}")
            nc.vector.tensor_tensor(p_lo[:], pf[:], p_hi[:], op=ALU.subtract)
            ptl = pp2.tile([128, NKC // 2, 128], F16, tag=f"ptl{tag}")
            nc.sync.dma_start_transpose(ptl[:], p_lo[:])
            ptl_halves.append(ptl)
    negm = wkp.tile([128, 1], F32, tag=f"negm{tag}")
    nc.vector.tensor_tensor(negm[:], nm[0][:], nm[1][:], op=ALU.min)
    sh = []
    lw = [wkp.tile([128, 1], F32, name=f"lw{tag}{h}", tag=f"lw{tag}{h}") for h in range(2)]
    for h in range(2):
        d = wkp.tile([128, 1], F32, name=f"d{tag}{h}", tag=f"d{tag}{h}")
        nc.vector.tensor_tensor(d[:], negm[:], nm[h][:], op=ALU.subtract)  # m_h - m <= 0
        s = wkp.tile([128, 1], F32, name=f"sh{tag}{h}", tag=f"sh{tag}{h}")
        nc.scalar.activation(s[:], d[:], AF.Exp)
        sh.append(s)
        nc.vector.tensor_tensor(lw[h][:], ls[h][:], s[:], op=ALU.mult)
    lsum = wkp.tile([128, 1], F32, tag=f"lsum{tag}")
    nc.vector.tensor_tensor(lsum[:], lw[0][:], lw[1][:], op=ALU.add)
    rl = wkp.tile([128, 1], F32, tag=f"rl{tag}")
    nc.vector.reciprocal(rl[:], lsum[:])
    c = []
    for h in range(2):
        ch = wkp.tile([128, 1], F32, name=f"c{tag}{h}", tag=f"c{tag}{h}")
        nc.vector.tensor_tensor(ch[:], sh[h][:], rl[:], op=ALU.mult)
        c.append(ch)
    return pth_halves, ptl_halves, c


def _build():
    import concourse.tile_utils as tile_utils
    tile_utils.max_sbuf_usage = 204 * 1024

    nc = bass.Bass("TRN2", target_bir_lowering=False, debug=False)

    xqh = nc.dram_tensor("xqh", [D1, SH], F16, kind="ExternalInput")
    yqh = nc.dram_tensor("yqh", [D1, SH], F16, kind="ExternalInput")
    xq8 = nc.dram_tensor("xq8", [D1, SH], F8, kind="ExternalInput")
    yq8 = nc.dram_tensor("yq8", [D1, SH], F8, kind="ExternalInput")
    wp32 = nc.dram_tensor("wp32", [PC32, 1024], F32, kind="ExternalInput")
    wp16 = nc.dram_tensor("wp16", [PC16, 1024], F16, kind="ExternalInput")
    out = nc.dram_tensor("out", [SH, D2], F16, kind="ExternalOutput")

    # collective bounce buffers (collectives can't touch External tensors)
    xbh = nc.dram_tensor("xbh", [D1, SH], F16)
    ybh = nc.dram_tensor("ybh", [D1, SH], F16)
    xb8 = nc.dram_tensor("xb8", [D1, SH], F8)
    yb8 = nc.dram_tensor("yb8", [D1, SH], F8)
    w32b = nc.dram_tensor("w32b", [PC32, 1024], F32)
    w16b = nc.dram_tensor("w16b", [PC16, 1024], F16)
    # gathered: rows [h*512:(h+1)*512] = x^T cols [h*1024:(h+1)*1024]
    xgh = nc.dram_tensor("xgh", [2 * D1, SH], F16)
    ygh = nc.dram_tensor("ygh", [2 * D1, SH], F16)
    xg8 = nc.dram_tensor("xg8", [2 * D1, SH], F8)
    yg8 = nc.dram_tensor("yg8", [2 * D1, SH], F8)
    w32g = nc.dram_tensor("w32g", [PR32, 1024], F32, addr_space="Shared")
    w16g = nc.dram_tensor("w16g", [PR16, 1024], F16, addr_space="Shared")

    # my tempT half (stage-1 outputs, transposed: [D1, my 1024 q cols])
    x1h_d = nc.dram_tensor("x1h_d", [D1, SH], F16)
    x1l_d = nc.dram_tensor("x1l_d", [D1, SH], F16)
    y1h_d = nc.dram_tensor("y1h_d", [D1, SH], F16)
    y1l_d = nc.dram_tensor("y1l_d", [D1, SH], F16)
    # pair-gathered tempT: rows [h*512:(h+1)*512] = cols [h*1024:(h+1)*1024]
    tgxh = nc.dram_tensor("tgxh", [2 * D1, SH], F16)
    tgxl = nc.dram_tensor("tgxl", [2 * D1, SH], F16)
    tgyh = nc.dram_tensor("tgyh", [2 * D1, SH], F16)
    tgyl = nc.dram_tensor("tgyl", [2 * D1, SH], F16)

    def cc(groups, i, o):
        nc.gpsimd.collective_compute(
            "AllGather", ALU.bypass, replica_groups=groups,
            ins=[i[:].opt()], outs=[o[:].opt()],
        )

    with TileContext(nc) as tc:
        for b_, s_ in [(xbh, xqh), (ybh, yqh), (xb8, xq8), (yb8, yq8),
                       (w32b, wp32), (w16b, wp16)]:
            nc.gpsimd.dma_start(out=b_[:], in_=s_[:])
        cc(PAIRS, xbh, xgh)
        cc(PAIRS, ybh, ygh)
        cc(PAIRS, xb8, xg8)
        cc(PAIRS, yb8, yg8)
        cc(FULL, w32b, w32g)
        cc(FULL, w16b, w16g)

        # payload views
        v512 = w32g.rearrange("r (k c) -> (r k) c", k=2)       # [5648, 512]
        vp8 = w32g.rearrange("r (p j) -> p r j", p=128)        # [128, 2824, 8]

        with tc.tile_pool(name="const", bufs=1) as cp:
            b1q_sb = cp.tile([128, 1, 8], F32, tag="b1q")
            b1k_sb = cp.tile([128, 1, 8], F32, tag="b1k")
            b2q_sb = cp.tile([128, 1, 8], F32, tag="b2q")
            b2k_sb = cp.tile([128, 1, 8], F32, tag="b2k")
            wres_sb = cp.tile([128, 1, 8], F32, tag="wres")
            for sb, row in [(b1q_sb, OB1Q), (b1k_sb, OB1K), (b2q_sb, OB2Q),
                            (b2k_sb, OB2K), (wres_sb, OWRES)]:
                nc.sync.dma_start(out=sb[:], in_=vp8[:, row:row + 1, :])
            b1v_sb = (cp.tile([1, D1], F16, name="b1vh", tag="b1vh"),
                      cp.tile([1, D1], F16, name="b1vl", tag="b1vl"))
            b2v_sb = (cp.tile([1, D2], F16, name="b2vh", tag="b2vh"),
                      cp.tile([1, D2], F16, name="b2vl", tag="b2vl"))
            with tc.tile_pool(name="cwk", bufs=1) as cwk:
                b1v_f = cwk.tile([1, D1], F32, tag="b1vf")
                b2v_f = cwk.tile([1, D2], F32, tag="b2vf")
                nc.sync.dma_start(out=b1v_f[:], in_=w32g[OB1V:OB1V + 1, 0:D1])
                nc.sync.dma_start(out=b2v_f[:], in_=w32g[OB2V:OB2V + 1, :])
                for (th, tl), tf in [(b1v_sb, b1v_f), (b2v_sb, b2v_f)]:
                    nc.vector.tensor_copy(th[:], tf[:])
                    nc.vector.tensor_tensor(tl[:], tf[:], th[:], op=ALU.subtract)
            ones_sb = cp.tile([1, 128], F16, tag="ones1")
            nc.vector.memset(ones_sb[:], 1.0)

            # ---------------- stage 1 ----------------
            with tc.tile_pool(name="loc", bufs=1) as locp:
                with tc.tile_pool(name="lwk", bufs=2) as lwk:
                    # W1^T chunk pairs [t][dc][128, 512]
                    w1sb = {}
                    for t, off in [("q", OW1Q), ("k", OW1K), ("v", OW1V)]:
                        w1sb[t] = _split_rows(
                            nc, locp, lwk,
                            lambda i, o=off: v512[2 * o + i * 128:2 * o + (i + 1) * 128, :],
                            D1, D1, f"w1{t}")
                    # local q-half pairs [dc][128, 1024]
                    xql = _hilo_rows(nc, locp, lwk,
                                     lambda i: xqh[i * 128:(i + 1) * 128, :],
                                     lambda i: xq8[i * 128:(i + 1) * 128, :],
                                     D1, SH, "xql")
                    yql = _hilo_rows(nc, locp, lwk,
                                     lambda i: yqh[i * 128:(i + 1) * 128, :],
                                     lambda i: yq8[i * 128:(i + 1) * 128, :],
                                     D1, SH, "yql")

                for ti, (src_g, q_loc, r_loc, wcol, o_hi, o_lo) in enumerate([
                        ((xgh, xg8), xql, yql, 0, x1h_d, x1l_d),
                        ((ygh, yg8), yql, xql, 1, y1h_d, y1l_d)]):
                    _stage1_attn(nc, tc, ti, src_g, q_loc, r_loc, wcol, o_hi, o_lo,
                                 w1sb, b1q_sb, b1k_sb, b1v_sb, ones_sb, wres_sb)

            cc(PAIRS, x1h_d, tgxh)
            cc(PAIRS, y1h_d, tgyh)
            cc(PAIRS, x1l_d, tgxl)
            cc(PAIRS, y1l_d, tgyl)

            # ---------------- stage 2 ----------------
            _stage2(nc, tc, (tgxh, tgxl, tgyh, tgyl),
                    (x1h_d, x1l_d, y1h_d, y1l_d), w32g, w16g,
                    b2q_sb, b2k_sb, b2v_sb, ones_sb, out)

    _fix_excess_waits(nc)
    return nc


def _stage1_attn(nc, tc, ti, src_g, q_loc, r_loc, wcol, o_hi, o_lo,
                 w1sb, b1q_sb, b1k_sb, b1v_sb, ones_sb, wres_sb):
    q_hi_loc, q_lo_loc = q_loc
    r_hi_loc, r_lo_loc = r_loc
    with (tc.tile_pool(name=f"kv{ti}", bufs=1) as kvp,
          tc.tile_pool(name=f"wk{ti}", bufs=2) as wkp,
          tc.tile_pool(name=f"ps{ti}", bufs=4, space="PSUM") as pp,
          tc.tile_pool(name=f"sc{ti}", bufs=2, space="PSUM") as scp):
        kt_hi = [kvp.tile([128, S], F16, name=f"kth{ec}", tag=f"kth{ec}")
                 for ec in range(NC1)]
        kt_lo = [kvp.tile([128, S], F16, name=f"ktl{ec}", tag=f"ktl{ec}")
                 for ec in range(NC1)]
        v_hi = [kvp.tile([128, D1], F16, name=f"vh{kc}", tag=f"vh{kc}")
                for kc in range(NKC)]
        v_lo = [kvp.tile([128, D1], F16, name=f"vl{kc}", tag=f"vl{kc}")
                for kc in range(NKC)]

        # full-sequence source pairs (scoped: freed before the q loop)
        with tc.tile_pool(name=f"src{ti}", bufs=1) as srcp, \
             tc.tile_pool(name=f"swk{ti}", bufs=2) as swk:
            src_gh, src_g8 = src_g
            src_hi, src_lo = [], []
            for dc in range(NC1):
                th = srcp.tile([128, S], F16, name=f"sfh{dc}", tag=f"sfh{dc}")
                tl = srcp.tile([128, S], F16, name=f"sfl{dc}", tag=f"sfl{dc}")
                for hh in range(2):
                    rsl = slice(hh * D1 + dc * 128, hh * D1 + (dc + 1) * 128)
                    csl = slice(hh * SH, (hh + 1) * SH)
                    nc.sync.dma_start(out=th[:, csl], in_=src_gh[rsl, :])
                    st8 = swk.tile([128, SH], F8, tag="st8_src")
                    nc.sync.dma_start(out=st8[:], in_=src_g8[rsl, :])
                    nc.vector.tensor_scalar(tl[:, csl], st8[:], 1.0 / LSCALE,
                                            None, op0=ALU.mult)
                src_hi.append(th)
                src_lo.append(tl)

            # K^T pair [ec][128, S]
            for ec in range(NC1):
                for sc in range(NSC):
                    ssl = slice(sc * 512, (sc + 1) * 512)
                    ps = pp.tile([128, 512], F32, tag="ps")
                    for dc in range(NC1):
                        _pair_mms(nc, ps[:],
                                  (w1sb["k"][0][dc][:, ec * 128:(ec + 1) * 128],
                                   w1sb["k"][1][dc][:, ec * 128:(ec + 1) * 128]),
                                  (src_hi[dc][:, ssl], src_lo[dc][:, ssl]),
                                  start=(dc == 0))
                    kf = wkp.tile([128, 512], F32, tag="kevac")
                    nc.vector.tensor_scalar(kf[:], ps[:], b1k_sb[:, 0, ec:ec + 1],
                                            None, op0=ALU.add)
                    nc.vector.tensor_copy(kt_hi[ec][:, ssl], kf[:])
                    nc.vector.tensor_tensor(kt_lo[ec][:, ssl], kf[:], kt_hi[ec][:, ssl],
                                            op=ALU.subtract)

            # V pair [kc][128, D1]; bias via rank-1 ones x b1v
            for kc in range(NKC):
                ps = pp.tile([128, 512], F32, tag="ps")
                nc.tensor.matmul(ps[:], ones_sb[:], b1v_sb[0][:], start=True, stop=False)
                nc.tensor.matmul(ps[:], ones_sb[:], b1v_sb[1][:], start=False, stop=False)
                for dc in range(NC1):
                    _pair_mms(nc, ps[:],
                              (src_hi[dc][:, kc * 128:(kc + 1) * 128],
                               src_lo[dc][:, kc * 128:(kc + 1) * 128]),
                              (w1sb["v"][0][dc][:], w1sb["v"][1][dc][:]),
                              start=False)
                nc.vector.tensor_copy(v_hi[kc][:], ps[:])
                nc.vector.tensor_tensor(v_lo[kc][:], ps[:], v_hi[kc][:], op=ALU.subtract)

        with (tc.tile_pool(name=f"pa{ti}", bufs=1) as ptp1,
              tc.tile_pool(name=f"pt{ti}", bufs=2) as ptp2):
            _stage1_qloop(nc, qi_pool=(ptp1, ptp2, pp, scp, wkp), w1sb=w1sb,
                          q_loc=(q_hi_loc, q_lo_loc), r_loc=(r_hi_loc, r_lo_loc),
                          kt=(kt_hi, kt_lo), v=(v_hi, v_lo), wcol=wcol,
                          b1q_sb=b1q_sb, wres_sb=wres_sb, o_hi=o_hi, o_lo=o_lo)


def _stage1_qloop(nc, qi_pool, w1sb, q_loc, r_loc, kt, v, wcol,
                  b1q_sb, wres_sb, o_hi, o_lo):
    ptp1, ptp2, pp, scp, wkp = qi_pool
    q_hi_loc, q_lo_loc = q_loc
    r_hi_loc, r_lo_loc = r_loc
    kt_hi, kt_lo = kt
    v_hi, v_lo = v
    if True:
        for qi in range(NQ):
            qsl = slice(qi * QT, (qi + 1) * QT)
            # Q^T for this tile from LOCAL half: psum [128, 4*128]
            qps = pp.tile([128, 512], F32, tag="ps")
            for ec in range(NC1):
                for dc in range(NC1):
                    _pair_mms(nc, qps[:, ec * 128:(ec + 1) * 128],
                              (w1sb["q"][0][dc][:, ec * 128:(ec + 1) * 128],
                               w1sb["q"][1][dc][:, ec * 128:(ec + 1) * 128]),
                              (q_hi_loc[dc][:, qsl], q_lo_loc[dc][:, qsl]),
                              start=(dc == 0))
            qf = wkp.tile([128, 512], F32, tag="qevac")
            for ec in range(NC1):
                esl = slice(ec * 128, (ec + 1) * 128)
                nc.vector.tensor_scalar(qf[:, esl], qps[:, esl],
                                        b1q_sb[:, 0, ec:ec + 1], None, op0=ALU.add)
            q_hi = wkp.tile([128, 512], F16, tag="qhi")
            q_lo = wkp.tile([128, 512], F16, tag="qlo")
            nc.vector.tensor_copy(q_hi[:], qf[:])
            nc.vector.tensor_tensor(q_lo[:], qf[:], q_hi[:], op=ALU.subtract)

            sps_h = [scp.tile([128, S // 2], F32, name=f"scr{h}", tag="scoresh")
                     for h in range(2)]
            for sc in range(NSC):
                ssl = slice(sc * 512, (sc + 1) * 512)
                hsl = slice((sc % 2) * 512, (sc % 2) * 512 + 512)
                for ec in range(NC1):
                    esl = slice(ec * 128, (ec + 1) * 128)
                    _pair_mms(nc, sps_h[sc // 2][:, hsl],
                              (q_hi[:, esl], q_lo[:, esl]),
                              (kt_hi[ec][:, ssl], kt_lo[ec][:, ssl]),
                              start=(ec == 0))

            pth, ptl, c = _softmax_ptiles(nc, ptp1, ptp2, wkp, sps_h, "1", pair=True)

            ops_h = []
            for h in range(2):
                ops = pp.tile([128, 512], F32, name=f"av{h}", tag="ps")
                for kc8 in range(NKC // 2):
                    kc = h * (NKC // 2) + kc8
                    nc.tensor.matmul(ops[:], pth[h][:, kc8, :], v_hi[kc][:],
                                     start=(kc8 == 0), stop=False)
                    nc.tensor.matmul(ops[:], pth[h][:, kc8, :], v_lo[kc][:],
                                     start=False, stop=False)
                    nc.tensor.matmul(ops[:], ptl[h][:, kc8, :], v_hi[kc][:],
                                     start=False, stop=(kc8 == NKC // 2 - 1))
                ops_h.append(ops)

            af = ptp1.tile([128, 512], F32, tag="af")
            nc.vector.tensor_scalar(af[:], ops_h[0][:], c[0][:, 0:1], None, op0=ALU.mult)
            af2 = ptp1.tile([128, 512], F32, tag="af2")
            nc.vector.tensor_scalar(af2[:], ops_h[1][:], c[1][:, 0:1], None, op0=ALU.mult)
            nc.vector.tensor_tensor(af[:], af[:], af2[:], op=ALU.add)
            a_hi = wkp.tile([128, 512], F16, tag="ahi")
            a_lo = wkp.tile([128, 512], F16, tag="alo")
            nc.scalar.copy(a_hi[:], af[:])
            nc.vector.tensor_tensor(a_lo[:], af[:], a_hi[:], op=ALU.subtract)
            at_hi = wkp.tile([128, NC1, 128], F16, tag="athi")
            at_lo = wkp.tile([128, NC1, 128], F16, tag="atlo")
            nc.sync.dma_start_transpose(at_hi[:], a_hi[:])
            nc.sync.dma_start_transpose(at_lo[:], a_lo[:])

            # residual in transposed space, then resplit; single strided store
            x1h = wkp.tile([128, NC1, 128], F16, tag="x1h")
            x1l = wkp.tile([128, NC1, 128], F16, tag="x1l")
            for ec in range(NC1):
                r1 = wkp.tile([128, 128], F32, tag="r1")
                nc.vector.tensor_scalar(r1[:], r_hi_loc[ec][:, qsl],
                                        wres_sb[:, 0, wcol:wcol + 1], None, op0=ALU.mult)
                nc.vector.tensor_tensor(r1[:], r1[:], at_hi[:, ec, :], op=ALU.add)
                r2 = wkp.tile([128, 128], F32, tag="r2")
                nc.vector.tensor_scalar(r2[:], r_lo_loc[ec][:, qsl],
                                        wres_sb[:, 0, wcol:wcol + 1], None, op0=ALU.mult)
                nc.vector.tensor_tensor(r2[:], r2[:], at_lo[:, ec, :], op=ALU.add)
                nc.vector.tensor_tensor(r1[:], r1[:], r2[:], op=ALU.add)
                nc.scalar.copy(x1h[:, ec, :], r1[:])
                nc.vector.tensor_tensor(x1l[:, ec, :], r1[:], x1h[:, ec, :], op=ALU.subtract)
            oh_ap = o_hi.rearrange("(c p) q -> p c q", p=128)[:, :, qsl]
            ol_ap = o_lo.rearrange("(c p) q -> p c q", p=128)[:, :, qsl]
            nc.gpsimd.dma_start(out=oh_ap, in_=x1h[:])
            nc.gpsimd.dma_start(out=ol_ap, in_=x1l[:])


def _stage2(nc, tc, tg, tloc, w32g, w16g, b2q_sb, b2k_sb, b2v_sb, ones_sb, out):
    tgxh, tgxl, tgyh, tgyl = tg
    x1h_d, x1l_d, y1h_d, y1l_d = tloc

    def tt_gath(dc, hi, shalf):
        if dc < NC1:
            dr = tgxh if hi else tgxl
        else:
            dr = tgyh if hi else tgyl
        r = shalf * D1 + (dc % NC1) * 128
        return dr[r:r + 128, :]

    def tt_loc(dc, hi):
        if dc < NC1:
            dr = x1h_d if hi else x1l_d
        else:
            dr = y1h_d if hi else y1l_d
        r = (dc % NC1) * 128
        return dr[r:r + 128, :]

    with (tc.tile_pool(name="s2", bufs=1) as s2p,
          tc.tile_pool(name="s2wk", bufs=2) as wkp,
          tc.tile_pool(name="s2pa", bufs=1) as ptp1,
          tc.tile_pool(name="s2pt", bufs=2) as ptp2,
          tc.tile_pool(name="s2ps", bufs=2, space="PSUM") as pp,
          tc.tile_pool(name="s2sc", bufs=2, space="PSUM") as scp):
        # V2 single fp16 [kc][128, D2]; temp-lo dropped; bias via rank-1
        v2 = []
        with tc.tile_pool(name="w2vp", bufs=1) as wp, \
             tc.tile_pool(name="ttv", bufs=2) as ttp:
            wv = []
            for i in range(NC2):
                t = wp.tile([128, D2], F16, tag=f"w2v{i}")
                nc.gpsimd.dma_start(out=t[:], in_=w16g[i * 128:(i + 1) * 128, :])
                wv.append(t)
            for kcg in range(NKC // 4):
                lsl_g = slice((kcg % 2) * 512, (kcg % 2) * 512 + 512)
                tchunks = []
                for dc in range(NC2):
                    t = ttp.tile([128, 512], F16, tag=f"ttv{dc}")
                    nc.gpsimd.dma_start(out=t[:], in_=tt_gath(dc, True, kcg // 2)[:, lsl_g])
                    tchunks.append(t)
                for kcl in range(4):
                    kc = kcg * 4 + kcl
                    lsl = slice(kcl * 128, (kcl + 1) * 128)
                    vt = s2p.tile([128, D2], F16, name=f"v2_{kc}", tag=f"v2{kc}")
                    for e2c in range(2):
                        esl = slice(e2c * 512, (e2c + 1) * 512)
                        ps = pp.tile([128, 512], F32, tag="ps2")
                        nc.tensor.matmul(ps[:], ones_sb[:], b2v_sb[0][:, esl],
                                         start=True, stop=False)
                        nc.tensor.matmul(ps[:], ones_sb[:], b2v_sb[1][:, esl],
                                         start=False, stop=False)
                        for dc in range(NC2):
                            nc.tensor.matmul(ps[:], tchunks[dc][:, lsl], wv[dc][:, esl],
                                             start=False, stop=(dc == NC2 - 1))
                        nc.vector.tensor_copy(vt[:, esl], ps[:])
                    v2.append(vt)

        # K2^T pair [ec][128, S]; gathered tempT pair + W2k^T f32 streamed
        k2_hi = [s2p.tile([128, S], F16, name=f"k2h{ec}", tag=f"k2h{ec}") for ec in range(NC2)]
        k2_lo = [s2p.tile([128, S], F16, name=f"k2l{ec}", tag=f"k2l{ec}") for ec in range(NC2)]
        with tc.tile_pool(name="w2ks", bufs=1) as wks, \
             tc.tile_pool(name="ttk", bufs=1) as ttp, \
             tc.tile_pool(name="wst", bufs=2) as wst:
            for sc in range(NSC):
                ssl = slice(sc * 512, (sc + 1) * 512)
                lsl_g = slice((sc % 2) * 512, (sc % 2) * 512 + 512)
                tch, tcl = [], []
                for dc in range(NC2):
                    th = ttp.tile([128, 512], F16, tag=f"ttkh{dc}")
                    tl = ttp.tile([128, 512], F16, tag=f"ttkl{dc}")
                    nc.gpsimd.dma_start(out=th[:], in_=tt_gath(dc, True, sc // 2)[:, lsl_g])
                    nc.gpsimd.dma_start(out=tl[:], in_=tt_gath(dc, False, sc // 2)[:, lsl_g])
                    tch.append(th)
                    tcl.append(tl)
                for e2h in range(2):
                    wsl = slice(e2h * 512, (e2h + 1) * 512)
                    wrh, wrl = [], []
                    for dc in range(NC2):
                        st = wst.tile([128, 512], F32, tag="wkst")
                        nc.sync.dma_start(
                            out=st[:],
                            in_=w32g[OW2K + dc * 128:OW2K + (dc + 1) * 128, wsl])
                        wh = wks.tile([128, 512], F16, name=f"wkh{dc}", tag=f"wkh{dc}")
                        wl = wks.tile([128, 512], F16, name=f"wkl{dc}", tag=f"wkl{dc}")
                        nc.vector.tensor_copy(wh[:], st[:])
                        nc.vector.tensor_tensor(wl[:], st[:], wh[:], op=ALU.subtract)
                        wrh.append(wh)
                        wrl.append(wl)
                    for ecl in range(4):
                        ec = e2h * 4 + ecl
                        lsl = slice(ecl * 128, (ecl + 1) * 128)
                        ps = pp.tile([128, 512], F32, tag="ps2")
                        for dc in range(NC2):
                            _pair_mms(nc, ps[:],
                                      (wrh[dc][:, lsl], wrl[dc][:, lsl]),
                                      (tch[dc][:], tcl[dc][:]),
                                      start=(dc == 0))
                        kf = wkp.tile([128, 512], F32, tag="k2evac")
                        nc.vector.tensor_scalar(kf[:], ps[:], b2k_sb[:, 0, ec:ec + 1],
                                                None, op0=ALU.add)
                        nc.vector.tensor_copy(k2_hi[ec][:, ssl], kf[:])
                        nc.vector.tensor_tensor(k2_lo[ec][:, ssl], kf[:], k2_hi[ec][:, ssl],
                                                op=ALU.subtract)

        # Q2^T pair for my SH query columns, from LOCAL tempT
        q2_hi = [s2p.tile([128, SH], F16, name=f"q2h{ec}", tag=f"q2h{ec}") for ec in range(NC2)]
        q2_lo = [s2p.tile([128, SH], F16, name=f"q2l{ec}", tag=f"q2l{ec}") for ec in range(NC2)]
        with tc.tile_pool(name="w2qs", bufs=1) as wqs, \
             tc.tile_pool(name="ttq", bufs=1) as ttp, \
             tc.tile_pool(name="wsq", bufs=2) as wst:
            for sc in range(SH // 512):
                ssl = slice(sc * 512, (sc + 1) * 512)
                tch, tcl = [], []
                for dc in range(NC2):
                    th = ttp.tile([128, 512], F16, tag=f"ttqh{dc}")
                    tl = ttp.tile([128, 512], F16, tag=f"ttql{dc}")
                    nc.gpsimd.dma_start(out=th[:], in_=tt_loc(dc, True)[:, ssl])
                    nc.gpsimd.dma_start(out=tl[:], in_=tt_loc(dc, False)[:, ssl])
                    tch.append(th)
                    tcl.append(tl)
                for e2h in range(2):
                    wsl = slice(e2h * 512, (e2h + 1) * 512)
                    wrh, wrl = [], []
                    for dc in range(NC2):
                        st = wst.tile([128, 512], F32, tag="wqst")
                        nc.sync.dma_start(
                            out=st[:],
                            in_=w32g[OW2Q + dc * 128:OW2Q + (dc + 1) * 128, wsl])
                        wh = wqs.tile([128, 512], F16, name=f"wqh{dc}", tag=f"wqh{dc}")
                        wl = wqs.tile([128, 512], F16, name=f"wql{dc}", tag=f"wql{dc}")
                        nc.vector.tensor_copy(wh[:], st[:])
                        nc.vector.tensor_tensor(wl[:], st[:], wh[:], op=ALU.subtract)
                        wrh.append(wh)
                        wrl.append(wl)
                    for ecl in range(4):
                        ec = e2h * 4 + ecl
                        lsl = slice(ecl * 128, (ecl + 1) * 128)
                        ps = pp.tile([128, 512], F32, tag="ps2")
                        for dc in range(NC2):
                            _pair_mms(nc, ps[:],
                                      (wrh[dc][:, lsl], wrl[dc][:, lsl]),
                                      (tch[dc][:], tcl[dc][:]),
                                      start=(dc == 0))
                        qf = wkp.tile([128, 512], F32, tag="q2evac")
                        nc.vector.tensor_scalar(qf[:], ps[:], b2q_sb[:, 0, ec:ec + 1],
                                                None, op0=ALU.add)
                        nc.vector.tensor_copy(q2_hi[ec][:, ssl], qf[:])
                        nc.vector.tensor_tensor(q2_lo[ec][:, ssl], qf[:], q2_hi[ec][:, ssl],
                                                op=ALU.subtract)

        # attention over my 8 q-tiles
        for qi in range(NQ):
            qsl = slice(qi * QT, (qi + 1) * QT)
            sps_h = [scp.tile([128, S // 2], F32, name=f"s2scr{h}", tag="s2scoresh")
                     for h in range(2)]
            for sc in range(NSC):
                ssl = slice(sc * 512, (sc + 1) * 512)
                hsl = slice((sc % 2) * 512, (sc % 2) * 512 + 512)
                for ec in range(NC2):
                    _pair_mms(nc, sps_h[sc // 2][:, hsl],
                              (q2_hi[ec][:, qsl], q2_lo[ec][:, qsl]),
                              (k2_hi[ec][:, ssl], k2_lo[ec][:, ssl]),
                              start=(ec == 0))

            pth, _, c = _softmax_ptiles(nc, ptp1, ptp2, wkp, sps_h, "2", pair=False)

            ops_h = []
            for h in range(2):
                ops = pp.tile([128, D2], F32, name=f"av2{h}", tag="ps2")
                for e2c in range(2):
                    esl = slice(e2c * 512, (e2c + 1) * 512)
                    for kc8 in range(NKC // 2):
                        kc = h * (NKC // 2) + kc8
                        nc.tensor.matmul(ops[:, esl], pth[h][:, kc8, :], v2[kc][:, esl],
                                         start=(kc8 == 0), stop=(kc8 == NKC // 2 - 1))
                ops_h.append(ops)
            of = ptp1.tile([128, D2], F32, tag="of2")
            nc.vector.tensor_scalar(of[:], ops_h[0][:], c[0][:, 0:1], None, op0=ALU.mult)
            of2 = ptp1.tile([128, D2], F32, tag="of2b")
            nc.vector.tensor_scalar(of2[:], ops_h[1][:], c[1][:, 0:1], None, op0=ALU.mult)
            nc.vector.tensor_tensor(of[:], of[:], of2[:], op=ALU.add)
            o16 = ptp1.tile([128, D2], F16, tag="o16")
            nc.vector.tensor_copy(o16[:], of[:])
            nc.sync.dma_start(out=out[qsl, :], in_=o16[:])


def _row_p8(vec):
    """Pack vec (len n*128, n<=8) into a 1024-row as flat[p*8+j] = vec[j*128+p]."""
    n = len(vec) // 128
    row = np.zeros((128, 8), np.float32)
    row[:, :n] = np.asarray(vec, np.float32).reshape(n, 128).T
    return row.reshape(1024)


def _prep_payload(inputs):
    pay32 = np.zeros((PR32, 1024), np.float32)
    pay32[OW1Q:OW1Q + 256] = np.ascontiguousarray(
        np.asarray(inputs["sa1_Wq"], np.float32).T).reshape(256, 1024)
    pay32[OW1K:OW1K + 256] = np.ascontiguousarray(
        np.asarray(inputs["sa1_Wk"], np.float32).T).reshape(256, 1024)
    pay32[OW1V:OW1V + 256] = np.ascontiguousarray(
        np.asarray(inputs["sa1_Wv"], np.float32).T).reshape(256, 1024)
    pay32[OW2Q:OW2Q + 1024] = np.asarray(inputs["sa2_Wq"], np.float32).T
    pay32[OW2K:OW2K + 1024] = np.asarray(inputs["sa2_Wk"], np.float32).T
    pay32[OB1Q] = _row_p8(inputs["sa1_bq"])
    pay32[OB1K] = _row_p8(inputs["sa1_bk"])
    pay32[OB1V, 0:D1] = np.asarray(inputs["sa1_bv"], np.float32)
    pay32[OB2Q] = _row_p8(inputs["sa2_bq"])
    pay32[OB2K] = _row_p8(inputs["sa2_bk"])
    pay32[OB2V] = np.asarray(inputs["sa2_bv"], np.float32)
    w1v = float(np.asarray(inputs["weight1"]).reshape(-1)[0])
    w2v = float(np.asarray(inputs["weight2"]).reshape(-1)[0])
    wres = np.zeros((128, 8), np.float32)
    wres[:, 0] = w2v
    wres[:, 1] = w1v
    pay32[OWRES] = wres.reshape(1024)
    pay16 = np.ascontiguousarray(
        np.asarray(inputs["sa2_Wv"], np.float32).T).astype(np.float16)
    return pay32, pay16


def _qhilo(a):
    """x^T -> (f16 hi, f8 lo*LSCALE) pair."""
    import ml_dtypes
    hi = a.T.astype(np.float16)
    lo = ((a.T - hi.astype(np.float32)) * LSCALE).astype(ml_dtypes.float8_e4m3fn)
    return hi, lo


def _prep_inputs(inputs):
    x = np.asarray(inputs["x"], np.float32)
    y = np.asarray(inputs["y"], np.float32)
    pay32, pay16 = _prep_payload(inputs)
    xt = [_qhilo(x[b]) for b in range(B)]
    yt = [_qhilo(y[b]) for b in range(B)]
    in_maps = []
    for c in range(8):
        b, h = c // 2, c % 2
        csl = slice(h * SH, (h + 1) * SH)
        in_maps.append({
            "xqh": xt[b][0][:, csl],
            "xq8": xt[b][1][:, csl],
            "yqh": yt[b][0][:, csl],
            "yq8": yt[b][1][:, csl],
            "wp32": pay32[c * PC32:(c + 1) * PC32],
            "wp16": pay16[c * PC16:(c + 1) * PC16],
        })
    return in_maps


def kernel(**inputs):
    if "nc" not in _CACHED:
        _CACHED["nc"] = _build()
    nc = _CACHED["nc"]
    in_maps = _prep_inputs(inputs)
    import time as _time
    _t0 = _time.time()
    res = run_bass_kernel_spmd(nc, in_maps, list(range(8)))
    _CACHED["exec_wall"] = _time.time() - _t0
    _CACHED["last_res"] = res
    out = np.empty((B, S, D2), np.float32)
    for c in range(8):
        b, h = c // 2, c % 2
        out[b, h * SH:(h + 1) * SH, :] = res.results[c]["out"]
    return out
